# revision 21
# baseline (speedup 1.0000x reference)
"""Trainium2 Bass kernel for a 3-layer minLSTM-style NLP model.

Model (per reference):
  x = emb[ids]                                   (B,S,E) = (2,2048,512)
  3 x { xn = LN(x); gates = xn @ Ws.T + bs;
        f' = sig(f)/(sig(f)+sig(i)); i' = 1-f';
        v = i' * g(tilde), g(x) = max(x+0.5, sigmoid(x));
        h_t = f'_t h_{t-1} + v_t  (h_0 = 0.5);  x = h + x }
  xf = LN(x) * fln_w;  logits = xf @ fc_w.T + fc_b    (B,S,32000)

Sharding (8 cores, zero collectives):
  core c -> (batch b=c//4, seq chunk j=c%4 of 512 tokens). Each core runs a
  640-token window (128-token halo before its own 512) through the recurrent
  stack; the forget-product decays the unknown initial state to ~0 over the
  halo, and a per-core reset constant makes j==0 exact at the batch start.
  Each core computes logits for its own 512 tokens against the full vocab.

Key layout tricks:
  - LN affine (ln_w/ln_b) folded into the gate weights/biases on host, and
    fln_w folded into fc_w, so the device only applies (x-m)*rstd.
  - Per-token LN stats computed TRANSPOSED (tokens on partitions) via 1-row
    matmuls; rsqrt via magic-constant Newton on tiny [128,5] tiles; result
    transposed back and broadcast across partitions by the Pool engine.
  - Activations f16 everywhere; output logits written f16 and upcast on host.
"""

import sys

if "/opt/trn_rl_repo" not in sys.path:
    sys.path.insert(0, "/opt/trn_rl_repo")

import numpy as np

import concourse.bass as bass
import concourse.bacc as bacc
import concourse.tile as tile
from concourse import mybir
from concourse.bass import IndirectOffsetOnAxis
from concourse.bass_utils import run_bass_kernel_spmd
from concourse.masks import make_identity
from concourse import dve_ops as _dve_ops
from concourse.dve_spec import AluOp as _DAlu, Bin as _DBin, Spec as _DSpec, \
    Src0 as _DS0, Src1 as _DS1, C0 as _DC0, C1 as _DC1, lower as _dlower, \
    _has_src1 as _dhas_src1
from concourse.dve_uop import DveOpSpec as _DveOpSpec


def _make_frac_op():
    """Custom DVE op: out = in0 / (in0 + in1), one bit-trick seed + one
    Newton pass (~1.7e-3 rel err). Fuses the minLSTM gate normalization
    f' = sig(f)/(sig(f)+sig(i)) into a single DVE instruction."""
    name = "FRAC_SIG_FAST_ANT"
    for op in _dve_ops.OPS:
        if op.name == name:
            return op
    _z = _DS0 + _DS1
    _nz = _DBin(_DAlu.BITWISE_NOT, _z, _z)
    _y0 = _nz * _DC0
    _y1 = _y0 * (_DC1 - _z * _y0)

    def _ref(in0, in1, c0, c1, c2):
        z = in0.astype(np.float32) + in1.astype(np.float32)
        nz = (~z.view(np.int32)).view(np.float32)
        y0 = nz * np.float32(c0)
        y1 = (y0 * (np.float32(c1) - z * y0)).astype(np.float32)
        return in0.astype(np.float32) * y1

    spec = _DSpec(body=_DS0 * _y1, reference=_ref)
    row = max(_dve_ops._SUB_OPCODE_FOR_NAME.values()) + 1
    _dve_ops._SUB_OPCODE_FOR_NAME[name] = row
    shas = {}
    for ver in ("v3", "v4"):
        u = _dlower(spec, ver=ver)
        shas[ver] = _DveOpSpec(name=name, opcode=row, uops=u,
                               rd1_en=_dhas_src1(spec)).sha(ver)
    op = _dve_ops.DveOp(name, spec, subdim=False, uops_sha=shas,
                        perf_en={"v3": True, "v4": True})
    _dve_ops.OPS.append(op)
    _dve_ops.CUSTOM_DVE_SPECS[name] = spec
    return op


FRAC_OP = _make_frac_op()
FRAC_C0 = -0.23549792
FRAC_C1 = 2.0017324

F32 = mybir.dt.float32
F16 = mybir.dt.float16
I32 = mybir.dt.int32
AF = mybir.ActivationFunctionType
OP = mybir.AluOpType

# problem constants
B, S, V, H, L = 2, 2048, 32000, 512, 3
P = 128
KT = H // P            # 4 k-tiles over the H contraction dim
CHUNK = 512            # own tokens per core
HALO = 128             # speculative scan warmup tokens
W = HALO + CHUNK       # 640 window tokens per core
NG = W // P            # 5 embedding gather groups
NCH = [(0, 512), (512, 128)]   # window free-dim chunks (PSUM-bounded)
VC = 500               # vocab chunk for logits
N_CORES = 8
EPS = 1e-5
MAGIC2 = 0x1EF759DF    # rsqrt seed magic, pre-adjusted for hneg=-(var+eps)/2


def build_program(fcwb=9, psgb=4, psgrb=1, pstb=2, workb=2, wstb=2):
    nc = bacc.Bacc("TRN2", target_bir_lowering=False, debug=False,
                   enable_asserts=True, num_devices=N_CORES)

    idx_t = nc.dram_tensor("idx", [P, NG], I32, kind="ExternalInput").ap()
    emb_t = nc.dram_tensor("emb", [V, H], F16, kind="ExternalInput").ap()
    wsT_t = nc.dram_tensor("wsT", [L, KT, P, 3 * H], F16, kind="ExternalInput").ap()
    bsg_t = nc.dram_tensor("bsg", [P, L * 12], F32, kind="ExternalInput").ap()
    bshalf_t = nc.dram_tensor("bshalf", [P, L * 4], F32, kind="ExternalInput").ap()
    fcwt_t = nc.dram_tensor("fcwt", [25, P, 10, KT, P], F16, kind="ExternalInput").ap()
    fcb_t = nc.dram_tensor("fcb", [P, V // P], F32, kind="ExternalInput").ap()
    rst_t = nc.dram_tensor("rst", [P, 2], F32, kind="ExternalInput").ap()
    out_t = nc.dram_tensor("out", [V, CHUNK], F16, kind="ExternalOutput").ap()

    with tile.TileContext(nc) as tc:
        with tc.tile_pool(name="singles", bufs=1) as singles, \
             tc.tile_pool(name="persist", bufs=1) as persist, \
             tc.tile_pool(name="fcw", bufs=fcwb) as fcwp:

            # ---- constants / small inputs ----
            idx = singles.tile([P, NG], I32)
            nc.sync.dma_start(out=idx[:], in_=idx_t[:])
            bsg = singles.tile([P, L * 12], F32)
            nc.sync.dma_start(out=bsg[:], in_=bsg_t[:])
            bshalf = singles.tile([P, L * 4], F32)
            nc.sync.dma_start(out=bshalf[:], in_=bshalf_t[:])
            rst = singles.tile([P, 2], F32)
            nc.sync.dma_start(out=rst[:], in_=rst_t[:])
            fcb2 = singles.tile([P, V // P], F32)
            nc.sync.dma_start(out=fcb2[:], in_=fcb_t[:])
            ident16 = singles.tile([P, P], F16)
            make_identity(nc, ident16[:])
            ones16 = singles.tile([P, 1], F16)   # stats-reduce rhs
            nc.vector.memset(ones16[:], 1.0)

            # final activations (channel-major), consumed by phase C
            xf_bf = [persist.tile([P, CHUNK], F16, tag=f"xfbf{k}", name=f"xfbf{k}")
                     for k in range(KT)]

            with tc.tile_pool(name="xpool", bufs=2) as xpool, \
                 tc.tile_pool(name="wst", bufs=wstb) as wstp, \
                 tc.tile_pool(name="work", bufs=workb) as work, \
                 tc.tile_pool(name="scan", bufs=1) as scanp, \
                 tc.tile_pool(name="xnp", bufs=1) as xnp, \
                 tc.tile_pool(name="gath", bufs=1) as gathp, \
                 tc.tile_pool(name="bc", bufs=1) as bcp, \
                 tc.tile_pool(name="stat", bufs=1) as statp, \
                 tc.tile_pool(name="psg", bufs=psgb, space="PSUM") as psg, \
                 tc.tile_pool(name="pss", bufs=1, space="PSUM") as pss, \
                 tc.tile_pool(name="psgr", bufs=psgrb, space="PSUM") as psgr, \
                 tc.tile_pool(name="pst", bufs=pstb, space="PSUM") as pst:

                # ---- phase A: embedding gather + transpose to channel-major
                x = [xpool.tile([P, W], F16, tag=f"x{k}", name=f"xt{k}") for k in range(KT)]
                xgs = []
                for g in range(NG):
                    xg = gathp.tile([P, H], F16, tag=f"xg{g}", name=f"xg{g}")
                    nc.gpsimd.indirect_dma_start(
                        out=xg[:], out_offset=None, in_=emb_t[:],
                        in_offset=IndirectOffsetOnAxis(ap=idx[:, g:g + 1], axis=0),
                    )
                    xgs.append(xg)
                for g in range(NG):
                    xg = xgs[g]
                    for k in range(KT):
                        ptr = pst.tile([P, P], F16, tag="pstt", name="ptr")
                        nc.tensor.transpose(
                            out=ptr[:], in_=xg[:, k * P:(k + 1) * P],
                            identity=ident16[:])
                        eng = nc.vector if (g * KT + k) % 2 == 0 else nc.scalar
                        if eng is nc.vector:
                            nc.vector.tensor_copy(
                                out=x[k][:, g * P:(g + 1) * P], in_=ptr[:])
                        else:
                            nc.scalar.copy(
                                out=x[k][:, g * P:(g + 1) * P], in_=ptr[:])

                # ---- helper: transposed LN stats + rsqrt newton ----
                def ln_stats(xs, g0, ngr, tag):
                    """Per-token -mean*rstd and rstd for token groups
                    [g0, g0+ngr) of the window, returned as a [1, 2*ngr*P]
                    f16 row on partition 0: cols [0,ngr*P) = rstd,
                    [ngr*P, 2*ngr*P) = mr."""
                    psT = pss.tile([P, 8], F32, tag="psT", name="psT")
                    sums_b = statp.tile([P, ngr], F32, tag="sumb")
                    scr = statp.tile([P, P], F32, tag="ttrscr")
                    for g in range(ngr):
                        sl = slice((g0 + g) * P, (g0 + g + 1) * P)
                        for k in range(KT):
                            nc.tensor.matmul(
                                out=psT[:, g:g + 1], lhsT=xs[k][:, sl],
                                rhs=ones16[:],
                                start=(k == 0), stop=(k == KT - 1))
                        gram = psgr.tile([P, P], F32, tag="gram", name="gram")
                        for k in range(KT):
                            nc.tensor.matmul(
                                out=gram[:], lhsT=xs[k][:, sl],
                                rhs=xs[k][:, sl],
                                start=(k == 0), stop=(k == KT - 1))
                        # sum_x2 = diag(gram) via (gram * I) row-reduce
                        nc.vector.tensor_tensor(
                            out=scr[:], in0=gram[:], in1=ident16[:],
                            op=OP.mult)
                        nc.vector.tensor_reduce(
                            out=sums_b[:, g:g + 1], in_=scr[:], op=OP.add,
                            axis=mybir.AxisListType.X)
                    m2 = statp.tile([P, ngr], F32, tag="m2")
                    hneg = statp.tile([P, ngr], F32, tag="hneg")
                    y = statp.tile([P, ngr], F32, tag="y")
                    t = statp.tile([P, ngr], F32, tag="t")
                    rsmr = statp.tile([P, 2 * ngr], F16, tag="rsmr")
                    # m2 = (sum_x/(H*sqrt2))^2 = m^2/2
                    nc.scalar.activation(out=m2[:], in_=psT[:, 0:ngr],
                                         func=AF.Square,
                                         scale=1.0 / (H * np.sqrt(2.0)))
                    # hneg = m^2/2 - (sum_x2/(2H) + eps/2) = -(var+eps)/2
                    nc.vector.tensor_scalar(
                        out=hneg[:], in0=sums_b[:],
                        scalar1=0.5 / H, scalar2=EPS / 2,
                        op0=OP.mult, op1=OP.add)
                    nc.vector.tensor_sub(hneg[:], m2[:], hneg[:])
                    # rsqrt seed: y = -( (bits(hneg)>>1) - MAGIC2 )
                    nc.vector.tensor_scalar(
                        out=y[:].bitcast(I32), in0=hneg[:].bitcast(I32),
                        scalar1=1, scalar2=None,
                        op0=OP.arith_shift_right)
                    nc.vector.tensor_scalar(
                        out=y[:].bitcast(I32), in0=y[:].bitcast(I32),
                        scalar1=MAGIC2, scalar2=-1, op0=OP.subtract,
                        op1=OP.mult)
                    for _ in range(1):
                        nc.vector.tensor_mul(t[:], y[:], y[:])
                        nc.vector.tensor_mul(t[:], t[:], hneg[:])
                        nc.vector.scalar_tensor_tensor(
                            out=y[:], in0=t[:], scalar=1.5, in1=y[:],
                            op0=OP.add, op1=OP.mult)
                    nc.vector.tensor_copy(out=rsmr[:, 0:ngr], in_=y[:])
                    # mr = -(sum_x/H)*rstd
                    nc.vector.scalar_tensor_tensor(
                        out=rsmr[:, ngr:2 * ngr], in0=psT[:, 0:ngr],
                        scalar=-1.0 / H, in1=y[:], op0=OP.mult, op1=OP.mult)
                    # transpose each column separately so every row lands
                    # on partition 0 (partition_broadcast requirement)
                    rows = statp.tile([1, 2 * NG * P], F16, tag="rows",
                                      name="rows")
                    for q in range(2 * ngr):
                        ptrq = pst.tile([1, P], F16, tag="pstt", name="ptrq")
                        nc.tensor.transpose(out=ptrq[:], in_=rsmr[:, q:q + 1],
                                            identity=ident16[:])
                        if q % 2 == 0:
                            nc.vector.tensor_copy(
                                out=rows[0:1, q * P:(q + 1) * P], in_=ptrq[:])
                        else:
                            nc.scalar.copy(
                                out=rows[0:1, q * P:(q + 1) * P], in_=ptrq[:])
                    return rows

                # ---- phase B: L recurrent layers ----
                for l in range(L):
                    wst = wstp.tile([P, KT * 3 * H], F16, tag="wst")
                    for kk in range(KT):
                        nc.sync.dma_start(
                            out=wst[:, kk * 3 * H:(kk + 1) * 3 * H],
                            in_=wsT_t[l, kk])

                    rows = ln_stats(x, 0, NG, "b")

                    # broadcast rstd/mr across partitions (Pool engine)
                    rb = bcp.tile([P, W], F16, tag="rb")
                    mb = bcp.tile([P, W], F16, tag="mb")
                    for g in range(NG):
                        nc.gpsimd.partition_broadcast(
                            rb[:, g * P:(g + 1) * P],
                            rows[0:1, g * P:(g + 1) * P])
                        nc.gpsimd.partition_broadcast(
                            mb[:, g * P:(g + 1) * P],
                            rows[0:1, (NG + g) * P:(NG + g + 1) * P])

                    # xn = (x - m) * rstd  (affine folded into weights)
                    xn = [xnp.tile([P, W], F16, tag=f"xn{k}", name=f"xn{k}") for k in range(KT)]
                    for k in range(KT):
                        nc.vector.tensor_mul(xn[k][:], x[k][:], rb[:])
                        nc.vector.tensor_add(xn[k][:], xn[k][:], mb[:])

                    # --- gates GEMM + nonlinearities + scan ---
                    fp = [scanp.tile([P, W], F16, tag=f"fp{k}", name=f"fp{k}") for k in range(KT)]
                    vv = [scanp.tile([P, W], F16, tag=f"vv{k}", name=f"vv{k}") for k in range(KT)]
                    hh = [scanp.tile([P, W], F16, tag=f"h{k}", name=f"h{k}") for k in range(KT)]
                    x2 = [xpool.tile([P, W], F16, tag=f"x{k}", name=f"xt{k}") for k in range(KT)]
                    for k in range(KT):
                        sf = work.tile([P, W], F16, tag="sf")
                        si = work.tile([P, W], F16, tag="si")
                        sg = work.tile([P, W], F16, tag="sg")
                        lin = work.tile([P, W], F16, tag="lin")
                        for (o, n) in NCH:
                            def gate_mm(gate):
                                pg = psg.tile([P, 512], F32, tag="pg")
                                for kk in range(KT):
                                    c0 = kk * 3 * H + gate * H + k * P
                                    nc.tensor.matmul(
                                        out=pg[:, :n],
                                        lhsT=(wst[:, c0:c0 + P]),
                                        rhs=(xn[kk][:, o:o + n]),
                                        start=(kk == 0), stop=(kk == KT - 1))
                                return pg

                            pg_f = gate_mm(0)
                            nc.scalar.activation(
                                out=sf[:, o:o + n], in_=pg_f[:, :n],
                                func=AF.Sigmoid,
                                bias=bsg[:, l * 12 + k:l * 12 + k + 1])
                            pg_i = gate_mm(1)
                            nc.scalar.activation(
                                out=si[:, o:o + n], in_=pg_i[:, :n],
                                func=AF.Sigmoid,
                                bias=bsg[:, l * 12 + 4 + k:l * 12 + 4 + k + 1])
                            pg_t = gate_mm(2)
                            nc.scalar.activation(
                                out=sg[:, o:o + n], in_=pg_t[:, :n],
                                func=AF.Sigmoid,
                                bias=bsg[:, l * 12 + 8 + k:l * 12 + 8 + k + 1])
                            nc.scalar.activation(
                                out=lin[:, o:o + n], in_=pg_t[:, :n],
                                func=AF.Identity,
                                bias=bshalf[:, l * 4 + k:l * 4 + k + 1])
                        # full-window gate math (one pass per k)
                        nc.vector._custom_dve(
                            FRAC_OP, out=fp[k][:], in0=sf[:], in1=si[:],
                            s0=FRAC_C0, s1=FRAC_C1)
                        g16 = work.tile([P, W], F16, tag="g16")
                        nc.vector.tensor_max(g16[:], lin[:], sg[:])
                        ip16 = work.tile([P, W], F16, tag="ip16")
                        nc.vector.tensor_scalar(
                            out=ip16[:], in0=fp[k][:],
                            scalar1=-1.0, scalar2=1.0,
                            op0=OP.mult, op1=OP.add)
                        nc.vector.tensor_mul(vv[k][:], ip16[:], g16[:])
                        # boundary reset at own-region start (exact for j==0)
                        t1 = work.tile([P, 1], F32, tag="t1")
                        nc.vector.tensor_mul(
                            t1[:], fp[k][:, HALO:HALO + 1], rst[:, 1:2])
                        nc.vector.tensor_add(
                            vv[k][:, HALO:HALO + 1], t1[:],
                            vv[k][:, HALO:HALO + 1])
                        nc.vector.tensor_mul(
                            fp[k][:, HALO:HALO + 1],
                            fp[k][:, HALO:HALO + 1], rst[:, 0:1])
                        nc.vector.tensor_tensor_scan(
                            out=hh[k][:], data0=fp[k][:], data1=vv[k][:],
                            initial=0.5, op0=OP.mult, op1=OP.add)
                        nc.vector.tensor_add(x2[k][:], hh[k][:], x[k][:])
                    x = x2

                # ---- final LayerNorm (own tokens = groups 1..4) ----
                rows2 = ln_stats(x, 1, NG - 1, "f")
                rb2 = bcp.tile([P, CHUNK], F16, tag="rb2")
                mb2 = bcp.tile([P, CHUNK], F16, tag="mb2")
                for g in range(NG - 1):
                    nc.gpsimd.partition_broadcast(
                        rb2[:, g * P:(g + 1) * P],
                        rows2[0:1, g * P:(g + 1) * P])
                    nc.gpsimd.partition_broadcast(
                        mb2[:, g * P:(g + 1) * P],
                        rows2[0:1, (NG - 1 + g) * P:(NG + g) * P])
                for k in range(KT):
                    nc.vector.tensor_mul(xf_bf[k][:], x[k][:, HALO:], rb2[:])
                    nc.vector.tensor_add(xf_bf[k][:], xf_bf[k][:], mb2[:])

            # ---- phase C: logits GEMM (own 512 tokens x full vocab) ----
            VG = 10   # vocab tiles per fcw load (25 groups of 10)
            with tc.tile_pool(name="osb", bufs=8) as osbp, \
                 tc.tile_pool(name="pso", bufs=8, space="PSUM") as pso:
                for vg in range(25):
                    fcw = fcwp.tile([P, VG, KT, P], F16, tag="fcw")
                    nc.gpsimd.dma_start(out=fcw[:], in_=fcwt_t[vg])
                    for j in range(VG):
                        vt = vg * VG + j
                        po = pso.tile([P, CHUNK], F32, tag="po")
                        for k in range(KT):
                            nc.tensor.matmul(
                                out=po[:], lhsT=fcw[:, j, k, :],
                                rhs=xf_bf[k][:],
                                start=(k == 0), stop=(k == KT - 1))
                        osb = osbp.tile([P, CHUNK], F16, tag="osb")
                        if vt % 2 == 0:
                            nc.scalar.activation(out=osb[:], in_=po[:],
                                                 func=AF.Identity,
                                                 bias=fcb2[:, vt:vt + 1])
                        else:
                            nc.vector.tensor_scalar(
                                out=osb[:], in0=po[:],
                                scalar1=fcb2[:, vt:vt + 1], scalar2=None,
                                op0=OP.add)
                        (nc.sync if vt % 2 == 0 else nc.scalar).dma_start(
                            out=out_t[vt * P:(vt + 1) * P, :], in_=osb[:])

    nc.compile()
    return nc


_CACHED = None


def _get_program():
    global _CACHED
    if _CACHED is None:
        _CACHED = build_program()
    return _CACHED


def prep_inputs(ids, emb, Ws, bs, ln_w, ln_b, fln_w, fc_w, fc_b):
    """Host-side layout prep -> per-core input maps."""
    ids = np.asarray(ids)
    emb = np.asarray(emb, dtype=np.float32)
    Ws = np.asarray(Ws, dtype=np.float32)
    bs = np.asarray(bs, dtype=np.float32)
    ln_w = np.asarray(ln_w, dtype=np.float32)
    ln_b = np.asarray(ln_b, dtype=np.float32)
    fln_w = np.asarray(fln_w, dtype=np.float32)
    fc_w = np.asarray(fc_w, dtype=np.float32)
    fc_b = np.asarray(fc_b, dtype=np.float32)

    emb16 = np.ascontiguousarray(emb).astype(np.float16)

    # fold ln_w into the gate weights, ln_b into the gate biases
    # Ws'[l] = Ws[l] * ln_w[l][None,:]; bias'[l] = bs[l] + Ws[l] @ ln_b[l]
    wsT = np.ascontiguousarray(
        np.stack([(Ws[l] * ln_w[l][None, :]).T.reshape(KT, P, 3 * H)
                  for l in range(L)])).astype(np.float16)
    bias = np.stack([bs[l] + Ws[l] @ ln_b[l] for l in range(L)])  # [L, 3H]

    # per-partition gate biases, grouped [l][gate][k]
    bsg = np.empty((P, L * 12), np.float32)
    bshalf = np.empty((P, L * 4), np.float32)
    for l in range(L):
        for gate in range(3):
            for k in range(KT):
                bsg[:, l * 12 + gate * 4 + k] = \
                    bias[l, gate * H + k * P:gate * H + (k + 1) * P]
        for k in range(KT):
            bshalf[:, l * 4 + k] = bias[l, 2 * H + k * P:2 * H + (k + 1) * P] + 0.5

    # fold fln_w into fc_w; fc_w'.T tiled [25, 128, 10, KT, 128] f16
    fcw = fc_w * fln_w[None, :]
    fcwt = np.ascontiguousarray(
        fcw.T.reshape(KT, P, 25, 10, P).transpose(2, 1, 3, 0, 4)).astype(
            np.float16)
    fcb2 = np.ascontiguousarray(fc_b.reshape(V // P, P).T)

    shared = {"emb": emb16, "wsT": wsT, "bsg": bsg, "bshalf": bshalf,
              "fcwt": fcwt, "fcb": fcb2}

    in_maps = []
    for c in range(N_CORES):
        b, j = divmod(c, 4)
        own0 = j * CHUNK
        win = np.zeros(W, np.int32)
        if j == 0:
            win[HALO:] = ids[b, :CHUNK]
        else:
            win[:] = ids[b, own0 - HALO:own0 + CHUNK]
        idxt = np.ascontiguousarray(win.reshape(NG, P).T)
        rstc = np.empty((P, 2), np.float32)
        rstc[:, 0] = 0.0 if j == 0 else 1.0   # multiplies f at window pos HALO
        rstc[:, 1] = 0.5 if j == 0 else 0.0   # adds f*this to v at pos HALO
        in_maps.append({**shared, "idx": idxt, "rst": rstc})
    return in_maps


def kernel(ids, emb, Ws, bs, ln_w, ln_b, fln_w, fc_w, fc_b):
    nc = _get_program()
    in_maps = prep_inputs(ids, emb, Ws, bs, ln_w, ln_b, fln_w, fc_w, fc_b)
    res = run_bass_kernel_spmd(nc, in_maps, list(range(N_CORES)))
    out = np.empty((B, S, V), np.float32)
    for c in range(N_CORES):
        b, j = divmod(c, 4)
        out[b, j * CHUNK:(j + 1) * CHUNK, :] = \
            res.results[c]["out"].T.astype(np.float32)
    return out


# revision 23
# speedup vs baseline: 1.0104x; 1.0104x over previous
"""Trainium2 Bass kernel for a 3-layer minLSTM-style NLP model.

Model (per reference):
  x = emb[ids]                                   (B,S,E) = (2,2048,512)
  3 x { xn = LN(x); gates = xn @ Ws.T + bs;
        f' = sig(f)/(sig(f)+sig(i)); i' = 1-f';
        v = i' * g(tilde), g(x) = max(x+0.5, sigmoid(x));
        h_t = f'_t h_{t-1} + v_t  (h_0 = 0.5);  x = h + x }
  xf = LN(x) * fln_w;  logits = xf @ fc_w.T + fc_b    (B,S,32000)

Sharding (8 cores, zero collectives):
  core c -> (batch b=c//4, seq chunk j=c%4 of 512 tokens). Each core runs a
  640-token window (128-token halo before its own 512) through the recurrent
  stack; the forget-product decays the unknown initial state to ~0 over the
  halo, and a per-core reset constant makes j==0 exact at the batch start.
  Each core computes logits for its own 512 tokens against the full vocab.

Key layout tricks:
  - LN affine (ln_w/ln_b) folded into the gate weights/biases on host, and
    fln_w folded into fc_w, so the device only applies (x-m)*rstd.
  - Per-token LN stats computed TRANSPOSED (tokens on partitions) via 1-row
    matmuls; rsqrt via magic-constant Newton on tiny [128,5] tiles; result
    transposed back and broadcast across partitions by the Pool engine.
  - Activations f16 everywhere; output logits written f16 and upcast on host.
"""

import sys

if "/opt/trn_rl_repo" not in sys.path:
    sys.path.insert(0, "/opt/trn_rl_repo")

import numpy as np

import concourse.bass as bass
import concourse.bacc as bacc
import concourse.tile as tile
from concourse import mybir
from concourse.bass import IndirectOffsetOnAxis
from concourse.bass_utils import run_bass_kernel_spmd
from concourse.masks import make_identity
from concourse import dve_ops as _dve_ops
from concourse.dve_spec import AluOp as _DAlu, Bin as _DBin, Spec as _DSpec, \
    Src0 as _DS0, Src1 as _DS1, C0 as _DC0, C1 as _DC1, lower as _dlower, \
    _has_src1 as _dhas_src1
from concourse.dve_uop import DveOpSpec as _DveOpSpec


def _make_frac_op():
    """Custom DVE op: out = in0 / (in0 + in1), one bit-trick seed + one
    Newton pass (~1.7e-3 rel err). Fuses the minLSTM gate normalization
    f' = sig(f)/(sig(f)+sig(i)) into a single DVE instruction."""
    name = "FRAC_SIG_FAST_ANT"
    for op in _dve_ops.OPS:
        if op.name == name:
            return op
    _z = _DS0 + _DS1
    _nz = _DBin(_DAlu.BITWISE_NOT, _z, _z)
    _y0 = _nz * _DC0
    _y1 = _y0 * (_DC1 - _z * _y0)

    def _ref(in0, in1, c0, c1, c2):
        z = in0.astype(np.float32) + in1.astype(np.float32)
        nz = (~z.view(np.int32)).view(np.float32)
        y0 = nz * np.float32(c0)
        y1 = (y0 * (np.float32(c1) - z * y0)).astype(np.float32)
        return in0.astype(np.float32) * y1

    spec = _DSpec(body=_DS0 * _y1, reference=_ref)
    row = max(_dve_ops._SUB_OPCODE_FOR_NAME.values()) + 1
    _dve_ops._SUB_OPCODE_FOR_NAME[name] = row
    shas = {}
    for ver in ("v3", "v4"):
        u = _dlower(spec, ver=ver)
        shas[ver] = _DveOpSpec(name=name, opcode=row, uops=u,
                               rd1_en=_dhas_src1(spec)).sha(ver)
    op = _dve_ops.DveOp(name, spec, subdim=False, uops_sha=shas,
                        perf_en={"v3": True, "v4": True})
    _dve_ops.OPS.append(op)
    _dve_ops.CUSTOM_DVE_SPECS[name] = spec
    return op


FRAC_OP = _make_frac_op()
FRAC_C0 = -0.23549792
FRAC_C1 = 2.0017324

F32 = mybir.dt.float32
F16 = mybir.dt.float16
I32 = mybir.dt.int32
AF = mybir.ActivationFunctionType
OP = mybir.AluOpType

# problem constants
B, S, V, H, L = 2, 2048, 32000, 512, 3
P = 128
KT = H // P            # 4 k-tiles over the H contraction dim
CHUNK = 512            # own tokens per core
HALO = 128             # speculative scan warmup tokens
W = HALO + CHUNK       # 640 window tokens per core
NG = W // P            # 5 embedding gather groups
NCH = [(0, 512), (512, 128)]   # window free-dim chunks (PSUM-bounded)
VC = 500               # vocab chunk for logits
N_CORES = 8
EPS = 1e-5
MAGIC2 = 0x1EF759DF    # rsqrt seed magic, pre-adjusted for hneg=-(var+eps)/2


def build_program(fcwb=9, psgb=4, psgrb=1, pstb=2, workb=2, wstb=2):
    nc = bacc.Bacc("TRN2", target_bir_lowering=False, debug=False,
                   enable_asserts=True, num_devices=N_CORES)

    idx_t = nc.dram_tensor("idx", [P, NG], I32, kind="ExternalInput").ap()
    emb_t = nc.dram_tensor("emb", [V, H], F16, kind="ExternalInput").ap()
    wsT_t = nc.dram_tensor("wsT", [L, KT, P, 3 * H], F16, kind="ExternalInput").ap()
    bsg_t = nc.dram_tensor("bsg", [P, L * 12], F32, kind="ExternalInput").ap()
    bshalf_t = nc.dram_tensor("bshalf", [P, L * 4], F32, kind="ExternalInput").ap()
    fcwt_t = nc.dram_tensor("fcwt", [25, P, 10, KT, P], F16, kind="ExternalInput").ap()
    fcb_t = nc.dram_tensor("fcb", [P, V // P], F32, kind="ExternalInput").ap()
    wlr_t = nc.dram_tensor("wlr", [1, L * 12 * P], F16, kind="ExternalInput").ap()
    rst_t = nc.dram_tensor("rst", [P, 2], F32, kind="ExternalInput").ap()
    out_t = nc.dram_tensor("out", [V, CHUNK], F16, kind="ExternalOutput").ap()

    with tile.TileContext(nc) as tc:
        with tc.tile_pool(name="singles", bufs=1) as singles, \
             tc.tile_pool(name="persist", bufs=1) as persist, \
             tc.tile_pool(name="fcw", bufs=fcwb) as fcwp:

            # ---- constants / small inputs ----
            idx = singles.tile([P, NG], I32)
            nc.sync.dma_start(out=idx[:], in_=idx_t[:])
            bsg = singles.tile([P, L * 12], F32)
            nc.sync.dma_start(out=bsg[:], in_=bsg_t[:])
            bshalf = singles.tile([P, L * 4], F32)
            nc.sync.dma_start(out=bshalf[:], in_=bshalf_t[:])
            wlr = singles.tile([1, L * 12 * P], F16)
            nc.sync.dma_start(out=wlr[:], in_=wlr_t[:])
            rst = singles.tile([P, 2], F32)
            nc.sync.dma_start(out=rst[:], in_=rst_t[:])
            fcb2 = singles.tile([P, V // P], F32)
            nc.sync.dma_start(out=fcb2[:], in_=fcb_t[:])
            ident16 = singles.tile([P, P], F16)
            make_identity(nc, ident16[:])
            ones16 = singles.tile([P, 1], F16)   # stats-reduce rhs
            nc.vector.memset(ones16[:], 1.0)

            # final activations (channel-major), consumed by phase C
            xf_bf = [persist.tile([P, CHUNK], F16, tag=f"xfbf{k}", name=f"xfbf{k}")
                     for k in range(KT)]

            with tc.tile_pool(name="xpool", bufs=2) as xpool, \
                 tc.tile_pool(name="wst", bufs=wstb) as wstp, \
                 tc.tile_pool(name="work", bufs=workb) as work, \
                 tc.tile_pool(name="scan", bufs=1) as scanp, \
                 tc.tile_pool(name="xnp", bufs=1) as xnp, \
                 tc.tile_pool(name="gath", bufs=1) as gathp, \
                 tc.tile_pool(name="bc", bufs=1) as bcp, \
                 tc.tile_pool(name="stat", bufs=1) as statp, \
                 tc.tile_pool(name="psg", bufs=psgb, space="PSUM") as psg, \
                 tc.tile_pool(name="pss", bufs=1, space="PSUM") as pss, \
                 tc.tile_pool(name="psgr", bufs=psgrb, space="PSUM") as psgr, \
                 tc.tile_pool(name="pst", bufs=pstb, space="PSUM") as pst:

                # ---- phase A: embedding gather + transpose to channel-major
                x = [xpool.tile([P, W], F16, tag=f"x{k}", name=f"xt{k}") for k in range(KT)]
                xgs = []
                for g in range(NG):
                    xg = gathp.tile([P, H], F16, tag=f"xg{g}", name=f"xg{g}")
                    nc.gpsimd.indirect_dma_start(
                        out=xg[:], out_offset=None, in_=emb_t[:],
                        in_offset=IndirectOffsetOnAxis(ap=idx[:, g:g + 1], axis=0),
                    )
                    xgs.append(xg)
                for g in range(NG):
                    xg = xgs[g]
                    for k in range(KT):
                        ptr = pst.tile([P, P], F16, tag="pstt", name="ptr")
                        nc.tensor.transpose(
                            out=ptr[:], in_=xg[:, k * P:(k + 1) * P],
                            identity=ident16[:])
                        eng = nc.vector if (g * KT + k) % 2 == 0 else nc.scalar
                        if eng is nc.vector:
                            nc.vector.tensor_copy(
                                out=x[k][:, g * P:(g + 1) * P], in_=ptr[:])
                        else:
                            nc.scalar.copy(
                                out=x[k][:, g * P:(g + 1) * P], in_=ptr[:])

                # ---- helper: transposed LN stats + rsqrt newton ----
                def ln_stats(xs, g0, ngr, tag):
                    """Per-token -mean*rstd and rstd for token groups
                    [g0, g0+ngr) of the window, returned as a [1, 2*ngr*P]
                    f16 row on partition 0: cols [0,ngr*P) = rstd,
                    [ngr*P, 2*ngr*P) = mr."""
                    psT = pss.tile([P, 8], F32, tag="psT", name="psT")
                    sums_b = statp.tile([P, ngr], F32, tag="sumb")
                    scr = statp.tile([P, P], F32, tag="ttrscr")
                    for g in range(ngr):
                        sl = slice((g0 + g) * P, (g0 + g + 1) * P)
                        for k in range(KT):
                            nc.tensor.matmul(
                                out=psT[:, g:g + 1], lhsT=xs[k][:, sl],
                                rhs=ones16[:],
                                start=(k == 0), stop=(k == KT - 1))
                        gram = psgr.tile([P, P], F32, tag="gram", name="gram")
                        for k in range(KT):
                            nc.tensor.matmul(
                                out=gram[:], lhsT=xs[k][:, sl],
                                rhs=xs[k][:, sl],
                                start=(k == 0), stop=(k == KT - 1))
                        # sum_x2 = diag(gram) via (gram * I) row-reduce
                        nc.vector.tensor_tensor(
                            out=scr[:], in0=gram[:], in1=ident16[:],
                            op=OP.mult)
                        nc.vector.tensor_reduce(
                            out=sums_b[:, g:g + 1], in_=scr[:], op=OP.add,
                            axis=mybir.AxisListType.X)
                    m2 = statp.tile([P, ngr], F32, tag="m2")
                    hneg = statp.tile([P, ngr], F32, tag="hneg")
                    y = statp.tile([P, ngr], F32, tag="y")
                    t = statp.tile([P, ngr], F32, tag="t")
                    rsmr = statp.tile([P, 2 * ngr], F16, tag="rsmr")
                    # m2 = (sum_x/(H*sqrt2))^2 = m^2/2
                    nc.scalar.activation(out=m2[:], in_=psT[:, 0:ngr],
                                         func=AF.Square,
                                         scale=1.0 / (H * np.sqrt(2.0)))
                    # hneg = m^2/2 - (sum_x2/(2H) + eps/2) = -(var+eps)/2
                    nc.vector.tensor_scalar(
                        out=hneg[:], in0=sums_b[:],
                        scalar1=0.5 / H, scalar2=EPS / 2,
                        op0=OP.mult, op1=OP.add)
                    nc.vector.tensor_sub(hneg[:], m2[:], hneg[:])
                    # rsqrt seed: y = -( (bits(hneg)>>1) - MAGIC2 )
                    nc.vector.tensor_scalar(
                        out=y[:].bitcast(I32), in0=hneg[:].bitcast(I32),
                        scalar1=1, scalar2=None,
                        op0=OP.arith_shift_right)
                    nc.vector.tensor_scalar(
                        out=y[:].bitcast(I32), in0=y[:].bitcast(I32),
                        scalar1=MAGIC2, scalar2=-1, op0=OP.subtract,
                        op1=OP.mult)
                    for _ in range(1):
                        nc.vector.tensor_mul(t[:], y[:], y[:])
                        nc.vector.tensor_mul(t[:], t[:], hneg[:])
                        nc.vector.scalar_tensor_tensor(
                            out=y[:], in0=t[:], scalar=1.5, in1=y[:],
                            op0=OP.add, op1=OP.mult)
                    nc.vector.tensor_copy(out=rsmr[:, 0:ngr], in_=y[:])
                    # mr = -(sum_x/H)*rstd
                    nc.vector.scalar_tensor_tensor(
                        out=rsmr[:, ngr:2 * ngr], in0=psT[:, 0:ngr],
                        scalar=-1.0 / H, in1=y[:], op0=OP.mult, op1=OP.mult)
                    # transpose each column separately so every row lands
                    # on partition 0 (partition_broadcast requirement)
                    rows = statp.tile([1, 2 * NG * P], F16, tag="rows",
                                      name="rows")
                    for q in range(2 * ngr):
                        ptrq = pst.tile([1, P], F16, tag="pstt", name="ptrq")
                        nc.tensor.transpose(out=ptrq[:], in_=rsmr[:, q:q + 1],
                                            identity=ident16[:])
                        if q % 2 == 0:
                            nc.vector.tensor_copy(
                                out=rows[0:1, q * P:(q + 1) * P], in_=ptrq[:])
                        else:
                            nc.scalar.copy(
                                out=rows[0:1, q * P:(q + 1) * P], in_=ptrq[:])
                    return rows

                def pe_filler(n, rhs_tile):
                    """Keep the PE p-state ramp warm with dead matmuls into
                    a rotating PSUM tile nobody reads."""
                    for _ in range(n):
                        pg = psg.tile([P, 512], F32, tag="pg", name="fill")
                        nc.tensor.matmul(out=pg[:], lhsT=ident16[:],
                                         rhs=rhs_tile[:, 0:512],
                                         start=True, stop=True)

                # ---- phase B: L recurrent layers ----
                for l in range(L):
                    wst = wstp.tile([P, KT * 3 * H], F16, tag="wst")
                    for kk in range(KT):
                        nc.sync.dma_start(
                            out=wst[:, kk * 3 * H:(kk + 1) * 3 * H],
                            in_=wsT_t[l, kk])

                    rows = ln_stats(x, 0, NG, "b")
                    pe_filler(12, x[0])

                    # broadcast rstd across partitions (Pool engine); the
                    # -m*rstd mean correction is folded into the gate GEMM as
                    # a rank-1 update (wl x mr) instead of a broadcast.
                    rb = bcp.tile([P, W], F16, tag="rb")
                    for g in range(NG):
                        nc.gpsimd.partition_broadcast(
                            rb[:, g * P:(g + 1) * P],
                            rows[0:1, g * P:(g + 1) * P])

                    # xn = x * rstd
                    xn = [xnp.tile([P, W], F16, tag=f"xn{k}", name=f"xn{k}") for k in range(KT)]
                    for k in range(KT):
                        nc.vector.tensor_mul(xn[k][:], x[k][:], rb[:])
                    pe_filler(8, x[0])

                    # --- gates GEMM + nonlinearities + scan ---
                    fp = [scanp.tile([P, W], F16, tag=f"fp{k}", name=f"fp{k}") for k in range(KT)]
                    vv = [scanp.tile([P, W], F16, tag=f"vv{k}", name=f"vv{k}") for k in range(KT)]
                    hh = [scanp.tile([P, W], F16, tag=f"h{k}", name=f"h{k}") for k in range(KT)]
                    x2 = [xpool.tile([P, W], F16, tag=f"x{k}", name=f"xt{k}") for k in range(KT)]
                    for k in range(KT):
                        sf = work.tile([P, W], F16, tag="sf")
                        si = work.tile([P, W], F16, tag="si")
                        sg = work.tile([P, W], F16, tag="sg")
                        lin = work.tile([P, W], F16, tag="lin")
                        for (o, n) in NCH:
                            def gate_mm(gate):
                                pg = psg.tile([P, 512], F32, tag="pg")
                                for kk in range(KT):
                                    c0 = kk * 3 * H + gate * H + k * P
                                    nc.tensor.matmul(
                                        out=pg[:, :n],
                                        lhsT=(wst[:, c0:c0 + P]),
                                        rhs=(xn[kk][:, o:o + n]),
                                        start=(kk == 0), stop=False)
                                w0 = (l * 12 + gate * 4 + k) * P
                                nc.tensor.matmul(
                                    out=pg[:, :n],
                                    lhsT=wlr[0:1, w0:w0 + P],
                                    rhs=rows[0:1, NG * P + o:NG * P + o + n],
                                    start=False, stop=True)
                                return pg

                            pg_f = gate_mm(0)
                            nc.scalar.activation(
                                out=sf[:, o:o + n], in_=pg_f[:, :n],
                                func=AF.Sigmoid,
                                bias=bsg[:, l * 12 + k:l * 12 + k + 1])
                            pg_i = gate_mm(1)
                            nc.scalar.activation(
                                out=si[:, o:o + n], in_=pg_i[:, :n],
                                func=AF.Sigmoid,
                                bias=bsg[:, l * 12 + 4 + k:l * 12 + 4 + k + 1])
                            pg_t = gate_mm(2)
                            nc.scalar.activation(
                                out=sg[:, o:o + n], in_=pg_t[:, :n],
                                func=AF.Sigmoid,
                                bias=bsg[:, l * 12 + 8 + k:l * 12 + 8 + k + 1])
                            nc.scalar.activation(
                                out=lin[:, o:o + n], in_=pg_t[:, :n],
                                func=AF.Identity,
                                bias=bshalf[:, l * 4 + k:l * 4 + k + 1])
                        # full-window gate math (one pass per k)
                        nc.vector._custom_dve(
                            FRAC_OP, out=fp[k][:], in0=sf[:], in1=si[:],
                            s0=FRAC_C0, s1=FRAC_C1)
                        g16 = work.tile([P, W], F16, tag="g16")
                        nc.vector.tensor_max(g16[:], lin[:], sg[:])
                        ip16 = work.tile([P, W], F16, tag="ip16")
                        nc.vector.tensor_scalar(
                            out=ip16[:], in0=fp[k][:],
                            scalar1=-1.0, scalar2=1.0,
                            op0=OP.mult, op1=OP.add)
                        nc.vector.tensor_mul(vv[k][:], ip16[:], g16[:])
                        # boundary reset at own-region start (exact for j==0)
                        t1 = work.tile([P, 1], F32, tag="t1")
                        nc.vector.tensor_mul(
                            t1[:], fp[k][:, HALO:HALO + 1], rst[:, 1:2])
                        nc.vector.tensor_add(
                            vv[k][:, HALO:HALO + 1], t1[:],
                            vv[k][:, HALO:HALO + 1])
                        nc.vector.tensor_mul(
                            fp[k][:, HALO:HALO + 1],
                            fp[k][:, HALO:HALO + 1], rst[:, 0:1])
                        nc.vector.tensor_tensor_scan(
                            out=hh[k][:], data0=fp[k][:], data1=vv[k][:],
                            initial=0.5, op0=OP.mult, op1=OP.add)
                        nc.vector.tensor_add(x2[k][:], hh[k][:], x[k][:])
                    x = x2

                # ---- final LayerNorm (own tokens = groups 1..4) ----
                rows2 = ln_stats(x, 1, NG - 1, "f")
                pe_filler(14, x[0])
                rb2 = bcp.tile([P, CHUNK], F16, tag="rb2")
                mb2 = bcp.tile([P, CHUNK], F16, tag="mb2")
                for g in range(NG - 1):
                    nc.gpsimd.partition_broadcast(
                        rb2[:, g * P:(g + 1) * P],
                        rows2[0:1, g * P:(g + 1) * P])
                    nc.gpsimd.partition_broadcast(
                        mb2[:, g * P:(g + 1) * P],
                        rows2[0:1, (NG - 1 + g) * P:(NG + g) * P])
                for k in range(KT):
                    nc.vector.tensor_mul(xf_bf[k][:], x[k][:, HALO:], rb2[:])
                    nc.vector.tensor_add(xf_bf[k][:], xf_bf[k][:], mb2[:])

            # ---- phase C: logits GEMM (own 512 tokens x full vocab) ----
            VG = 10   # vocab tiles per fcw load (25 groups of 10)
            with tc.tile_pool(name="osb", bufs=8) as osbp, \
                 tc.tile_pool(name="pso", bufs=8, space="PSUM") as pso:
                for vg in range(25):
                    fcw = fcwp.tile([P, VG, KT, P], F16, tag="fcw")
                    nc.gpsimd.dma_start(out=fcw[:], in_=fcwt_t[vg])
                    for j in range(VG):
                        vt = vg * VG + j
                        po = pso.tile([P, CHUNK], F32, tag="po")
                        for k in range(KT):
                            nc.tensor.matmul(
                                out=po[:], lhsT=fcw[:, j, k, :],
                                rhs=xf_bf[k][:],
                                start=(k == 0), stop=(k == KT - 1))
                        osb = osbp.tile([P, CHUNK], F16, tag="osb")
                        if vt % 2 == 0:
                            nc.scalar.activation(out=osb[:], in_=po[:],
                                                 func=AF.Identity,
                                                 bias=fcb2[:, vt:vt + 1])
                        else:
                            nc.vector.tensor_scalar(
                                out=osb[:], in0=po[:],
                                scalar1=fcb2[:, vt:vt + 1], scalar2=None,
                                op0=OP.add)
                        (nc.sync if vt % 2 == 0 else nc.scalar).dma_start(
                            out=out_t[vt * P:(vt + 1) * P, :], in_=osb[:])

    nc.compile()
    return nc


_CACHED = None


def _get_program():
    global _CACHED
    if _CACHED is None:
        _CACHED = build_program()
    return _CACHED


def prep_inputs(ids, emb, Ws, bs, ln_w, ln_b, fln_w, fc_w, fc_b):
    """Host-side layout prep -> per-core input maps."""
    ids = np.asarray(ids)
    emb = np.asarray(emb, dtype=np.float32)
    Ws = np.asarray(Ws, dtype=np.float32)
    bs = np.asarray(bs, dtype=np.float32)
    ln_w = np.asarray(ln_w, dtype=np.float32)
    ln_b = np.asarray(ln_b, dtype=np.float32)
    fln_w = np.asarray(fln_w, dtype=np.float32)
    fc_w = np.asarray(fc_w, dtype=np.float32)
    fc_b = np.asarray(fc_b, dtype=np.float32)

    emb16 = np.ascontiguousarray(emb).astype(np.float16)

    # fold ln_w into the gate weights, ln_b into the gate biases
    # Ws'[l] = Ws[l] * ln_w[l][None,:]; bias'[l] = bs[l] + Ws[l] @ ln_b[l]
    wsT = np.ascontiguousarray(
        np.stack([(Ws[l] * ln_w[l][None, :]).T.reshape(KT, P, 3 * H)
                  for l in range(L)])).astype(np.float16)
    bias = np.stack([bs[l] + Ws[l] @ ln_b[l] for l in range(L)])  # [L, 3H]

    # gate-weight column sums (for the rank-1 mean correction), as a
    # partition-0 row grouped [l][gate][k][128]
    wl = np.stack([(Ws[l] * ln_w[l][None, :]).sum(axis=1) for l in range(L)])
    wlr = np.empty((1, L * 12 * P), np.float32)
    for l in range(L):
        for gate in range(3):
            for k in range(KT):
                c = (l * 12 + gate * 4 + k) * P
                wlr[0, c:c + P] = wl[l, gate * H + k * P:gate * H + (k + 1) * P]

    # per-partition gate biases, grouped [l][gate][k]
    bsg = np.empty((P, L * 12), np.float32)
    bshalf = np.empty((P, L * 4), np.float32)
    for l in range(L):
        for gate in range(3):
            for k in range(KT):
                bsg[:, l * 12 + gate * 4 + k] = \
                    bias[l, gate * H + k * P:gate * H + (k + 1) * P]
        for k in range(KT):
            bshalf[:, l * 4 + k] = bias[l, 2 * H + k * P:2 * H + (k + 1) * P] + 0.5

    # fold fln_w into fc_w; fc_w'.T tiled [25, 128, 10, KT, 128] f16
    fcw = fc_w * fln_w[None, :]
    fcwt = np.ascontiguousarray(
        fcw.T.reshape(KT, P, 25, 10, P).transpose(2, 1, 3, 0, 4)).astype(
            np.float16)
    fcb2 = np.ascontiguousarray(fc_b.reshape(V // P, P).T)

    shared = {"emb": emb16, "wsT": wsT, "bsg": bsg, "bshalf": bshalf,
              "wlr": wlr.astype(np.float16), "fcwt": fcwt, "fcb": fcb2}

    in_maps = []
    for c in range(N_CORES):
        b, j = divmod(c, 4)
        own0 = j * CHUNK
        win = np.zeros(W, np.int32)
        if j == 0:
            win[HALO:] = ids[b, :CHUNK]
        else:
            win[:] = ids[b, own0 - HALO:own0 + CHUNK]
        idxt = np.ascontiguousarray(win.reshape(NG, P).T)
        rstc = np.empty((P, 2), np.float32)
        rstc[:, 0] = 0.0 if j == 0 else 1.0   # multiplies f at window pos HALO
        rstc[:, 1] = 0.5 if j == 0 else 0.0   # adds f*this to v at pos HALO
        in_maps.append({**shared, "idx": idxt, "rst": rstc})
    return in_maps


def kernel(ids, emb, Ws, bs, ln_w, ln_b, fln_w, fc_w, fc_b):
    nc = _get_program()
    in_maps = prep_inputs(ids, emb, Ws, bs, ln_w, ln_b, fln_w, fc_w, fc_b)
    res = run_bass_kernel_spmd(nc, in_maps, list(range(N_CORES)))
    out = np.empty((B, S, V), np.float32)
    for c in range(N_CORES):
        b, j = divmod(c, 4)
        out[b, j * CHUNK:(j + 1) * CHUNK, :] = \
            res.results[c]["out"].T.astype(np.float32)
    return out


# revision 27
# speedup vs baseline: 1.0630x; 1.0520x over previous
"""Trainium2 Bass kernel for a 3-layer minLSTM-style NLP model.

Model (per reference):
  x = emb[ids]                                   (B,S,E) = (2,2048,512)
  3 x { xn = LN(x); gates = xn @ Ws.T + bs;
        f' = sig(f)/(sig(f)+sig(i)); i' = 1-f';
        v = i' * g(tilde), g(x) = max(x+0.5, sigmoid(x));
        h_t = f'_t h_{t-1} + v_t  (h_0 = 0.5);  x = h + x }
  xf = LN(x) * fln_w;  logits = xf @ fc_w.T + fc_b    (B,S,32000)

Sharding (8 cores, zero collectives):
  core c -> (batch b=c//4, seq chunk j=c%4 of 512 tokens). Each core runs a
  640-token window (128-token halo before its own 512) through the recurrent
  stack; the forget-product decays the unknown initial state to ~0 over the
  halo, and a per-core reset constant makes j==0 exact at the batch start.
  Each core computes logits for its own 512 tokens against the full vocab.

Key layout tricks:
  - LN affine (ln_w/ln_b) folded into the gate weights/biases on host, and
    fln_w folded into fc_w, so the device only applies (x-m)*rstd.
  - Per-token LN stats computed TRANSPOSED (tokens on partitions) via 1-row
    matmuls; rsqrt via magic-constant Newton on tiny [128,5] tiles; result
    transposed back and broadcast across partitions by the Pool engine.
  - Activations f16 everywhere; output logits written f16 and upcast on host.
"""

import sys

if "/opt/trn_rl_repo" not in sys.path:
    sys.path.insert(0, "/opt/trn_rl_repo")

import numpy as np

import concourse.bass as bass
import concourse.bacc as bacc
import concourse.tile as tile
from concourse import mybir
from concourse.bass import IndirectOffsetOnAxis
from concourse.bass_utils import run_bass_kernel_spmd
from concourse.masks import make_identity
from concourse import dve_ops as _dve_ops
from concourse.dve_spec import AluOp as _DAlu, Bin as _DBin, Spec as _DSpec, \
    Src0 as _DS0, Src1 as _DS1, C0 as _DC0, C1 as _DC1, lower as _dlower, \
    _has_src1 as _dhas_src1
from concourse.dve_uop import DveOpSpec as _DveOpSpec


def _make_frac_op():
    """Custom DVE op: out = in0 / (in0 + in1), one bit-trick seed + one
    Newton pass (~1.7e-3 rel err). Fuses the minLSTM gate normalization
    f' = sig(f)/(sig(f)+sig(i)) into a single DVE instruction."""
    name = "FRAC_SIG_FAST_ANT"
    for op in _dve_ops.OPS:
        if op.name == name:
            return op
    _z = _DS0 + _DS1
    _nz = _DBin(_DAlu.BITWISE_NOT, _z, _z)
    _y0 = _nz * _DC0
    _y1 = _y0 * (_DC1 - _z * _y0)

    def _ref(in0, in1, c0, c1, c2):
        z = in0.astype(np.float32) + in1.astype(np.float32)
        nz = (~z.view(np.int32)).view(np.float32)
        y0 = nz * np.float32(c0)
        y1 = (y0 * (np.float32(c1) - z * y0)).astype(np.float32)
        return in0.astype(np.float32) * y1

    spec = _DSpec(body=_DS0 * _y1, reference=_ref)
    row = max(_dve_ops._SUB_OPCODE_FOR_NAME.values()) + 1
    _dve_ops._SUB_OPCODE_FOR_NAME[name] = row
    shas = {}
    for ver in ("v3", "v4"):
        u = _dlower(spec, ver=ver)
        shas[ver] = _DveOpSpec(name=name, opcode=row, uops=u,
                               rd1_en=_dhas_src1(spec)).sha(ver)
    op = _dve_ops.DveOp(name, spec, subdim=False, uops_sha=shas,
                        perf_en={"v3": True, "v4": True})
    _dve_ops.OPS.append(op)
    _dve_ops.CUSTOM_DVE_SPECS[name] = spec
    return op


FRAC_OP = _make_frac_op()
FRAC_C0 = -0.23549792
FRAC_C1 = 2.0017324

F32 = mybir.dt.float32
F16 = mybir.dt.float16
F8 = mybir.dt.float8e4
I32 = mybir.dt.int32
AF = mybir.ActivationFunctionType
OP = mybir.AluOpType

# problem constants
B, S, V, H, L = 2, 2048, 32000, 512, 3
P = 128
KT = H // P            # 4 k-tiles over the H contraction dim
CHUNK = 512            # own tokens per core
HALO = 128             # speculative scan warmup tokens
W = HALO + CHUNK       # 640 window tokens per core
NG = W // P            # 5 embedding gather groups
NCH = [(0, 512), (512, 128)]   # window free-dim chunks (PSUM-bounded)
VC = 500               # vocab chunk for logits
N_CORES = 8
EPS = 1e-5
MAGIC2 = 0x1EF759DF    # rsqrt seed magic, pre-adjusted for hneg=-(var+eps)/2


def build_program(fcwb=9, psgb=4, psgrb=1, pstb=2, workb=2, wstb=2, fa=12, fb=8, fc=14):
    nc = bacc.Bacc("TRN2", target_bir_lowering=False, debug=False,
                   enable_asserts=True, num_devices=N_CORES)

    idx_t = nc.dram_tensor("idx", [P, NG], I32, kind="ExternalInput").ap()
    emb_t = nc.dram_tensor("emb", [V, H], F16, kind="ExternalInput").ap()
    wsT_t = nc.dram_tensor("wsT", [L, KT, P, 3 * H], F16, kind="ExternalInput").ap()
    bsg_t = nc.dram_tensor("bsg", [P, L * 12], F32, kind="ExternalInput").ap()
    bshalf_t = nc.dram_tensor("bshalf", [P, L * 4], F32, kind="ExternalInput").ap()
    fcwt_t = nc.dram_tensor("fcwt", [25, P, 10, 2, 2, 2, P], F8, kind="ExternalInput").ap()
    fcb_t = nc.dram_tensor("fcb", [P, V // P], F32, kind="ExternalInput").ap()
    wlr_t = nc.dram_tensor("wlr", [1, L * 12 * P], F16, kind="ExternalInput").ap()
    rst_t = nc.dram_tensor("rst", [P, 2], F32, kind="ExternalInput").ap()
    out_t = nc.dram_tensor("out", [V, CHUNK], F16, kind="ExternalOutput").ap()

    with tile.TileContext(nc) as tc:
        with tc.tile_pool(name="singles", bufs=1) as singles, \
             tc.tile_pool(name="persist", bufs=1) as persist, \
             tc.tile_pool(name="fcw", bufs=fcwb) as fcwp:

            # ---- constants / small inputs ----
            idx = singles.tile([P, NG], I32)
            nc.sync.dma_start(out=idx[:], in_=idx_t[:])
            bsg = singles.tile([P, L * 12], F32)
            nc.sync.dma_start(out=bsg[:], in_=bsg_t[:])
            bshalf = singles.tile([P, L * 4], F32)
            nc.sync.dma_start(out=bshalf[:], in_=bshalf_t[:])
            wlr = singles.tile([1, L * 12 * P], F16)
            nc.sync.dma_start(out=wlr[:], in_=wlr_t[:])
            rst = singles.tile([P, 2], F32)
            nc.sync.dma_start(out=rst[:], in_=rst_t[:])
            fcb2 = singles.tile([P, V // P], F32)
            nc.sync.dma_start(out=fcb2[:], in_=fcb_t[:])
            ident16 = singles.tile([P, P], F16)
            make_identity(nc, ident16[:])
            actwarm = singles.tile([1, 1], F32)
            nc.scalar.activation(out=actwarm[:], in_=rst[0:1, 0:1],
                                 func=AF.Sigmoid)
            ones16 = singles.tile([P, 1], F16)   # stats-reduce rhs
            nc.vector.memset(ones16[:], 1.0)

            # final activations (channel-major), consumed by phase C
            xf_bf = [persist.tile([P, CHUNK], F16, tag=f"xfbf{k}", name=f"xfbf{k}")
                     for k in range(KT)]

            with tc.tile_pool(name="xpool", bufs=2) as xpool, \
                 tc.tile_pool(name="wst", bufs=wstb) as wstp, \
                 tc.tile_pool(name="work", bufs=workb) as work, \
                 tc.tile_pool(name="scan", bufs=1) as scanp, \
                 tc.tile_pool(name="xnp", bufs=1) as xnp, \
                 tc.tile_pool(name="gath", bufs=1) as gathp, \
                 tc.tile_pool(name="bc", bufs=1) as bcp, \
                 tc.tile_pool(name="stat", bufs=1) as statp, \
                 tc.tile_pool(name="psg", bufs=psgb, space="PSUM") as psg, \
                 tc.tile_pool(name="pss", bufs=1, space="PSUM") as pss, \
                 tc.tile_pool(name="psgr", bufs=psgrb, space="PSUM") as psgr, \
                 tc.tile_pool(name="pst", bufs=pstb, space="PSUM") as pst:

                # ---- phase A: embedding gather + transpose to channel-major
                x = [xpool.tile([P, W], F16, tag=f"x{k}", name=f"xt{k}") for k in range(KT)]
                xgs = []
                for g in range(NG):
                    xg = gathp.tile([P, H], F16, tag=f"xg{g}", name=f"xg{g}")
                    nc.gpsimd.indirect_dma_start(
                        out=xg[:], out_offset=None, in_=emb_t[:],
                        in_offset=IndirectOffsetOnAxis(ap=idx[:, g:g + 1], axis=0),
                    )
                    xgs.append(xg)
                for g in range(NG):
                    xg = xgs[g]
                    for k in range(KT):
                        ptr = pst.tile([P, P], F16, tag="pstt", name="ptr")
                        nc.tensor.transpose(
                            out=ptr[:], in_=xg[:, k * P:(k + 1) * P],
                            identity=ident16[:])
                        eng = nc.vector if (g * KT + k) % 2 == 0 else nc.scalar
                        if eng is nc.vector:
                            nc.vector.tensor_copy(
                                out=x[k][:, g * P:(g + 1) * P], in_=ptr[:])
                        else:
                            nc.scalar.copy(
                                out=x[k][:, g * P:(g + 1) * P], in_=ptr[:])

                # ---- helper: transposed LN stats + rsqrt newton ----
                def ln_stats(xs, g0, ngr, tag):
                    """Per-token -mean*rstd and rstd for token groups
                    [g0, g0+ngr) of the window, returned as a [1, 2*ngr*P]
                    f16 row on partition 0: cols [0,ngr*P) = rstd,
                    [ngr*P, 2*ngr*P) = mr."""
                    psT = pss.tile([P, 8], F32, tag="psT", name="psT")
                    sums_b = statp.tile([P, ngr], F32, tag="sumb")
                    scr = statp.tile([P, P], F32, tag="ttrscr")
                    for g in range(ngr):
                        sl = slice((g0 + g) * P, (g0 + g + 1) * P)
                        for k in range(KT):
                            nc.tensor.matmul(
                                out=psT[:, g:g + 1], lhsT=xs[k][:, sl],
                                rhs=ones16[:],
                                start=(k == 0), stop=(k == KT - 1))
                        gram = psgr.tile([P, P], F32, tag="gram", name="gram")
                        for k in range(KT):
                            nc.tensor.matmul(
                                out=gram[:], lhsT=xs[k][:, sl],
                                rhs=xs[k][:, sl],
                                start=(k == 0), stop=(k == KT - 1))
                        # sum_x2 = diag(gram) via (gram * I) row-reduce
                        nc.vector.tensor_tensor(
                            out=scr[:], in0=gram[:], in1=ident16[:],
                            op=OP.mult)
                        nc.vector.tensor_reduce(
                            out=sums_b[:, g:g + 1], in_=scr[:], op=OP.add,
                            axis=mybir.AxisListType.X)
                    m2 = statp.tile([P, ngr], F32, tag="m2")
                    hneg = statp.tile([P, ngr], F32, tag="hneg")
                    y = statp.tile([P, ngr], F32, tag="y")
                    t = statp.tile([P, ngr], F32, tag="t")
                    rsmr = statp.tile([P, 2 * ngr], F16, tag="rsmr")
                    # m2 = (sum_x/(H*sqrt2))^2 = m^2/2
                    nc.scalar.activation(out=m2[:], in_=psT[:, 0:ngr],
                                         func=AF.Square,
                                         scale=1.0 / (H * np.sqrt(2.0)))
                    # hneg = m^2/2 - (sum_x2/(2H) + eps/2) = -(var+eps)/2
                    nc.vector.tensor_scalar(
                        out=hneg[:], in0=sums_b[:],
                        scalar1=0.5 / H, scalar2=EPS / 2,
                        op0=OP.mult, op1=OP.add)
                    nc.vector.tensor_sub(hneg[:], m2[:], hneg[:])
                    # rsqrt seed: y = -( (bits(hneg)>>1) - MAGIC2 )
                    nc.vector.tensor_scalar(
                        out=y[:].bitcast(I32), in0=hneg[:].bitcast(I32),
                        scalar1=1, scalar2=None,
                        op0=OP.arith_shift_right)
                    nc.vector.tensor_scalar(
                        out=y[:].bitcast(I32), in0=y[:].bitcast(I32),
                        scalar1=MAGIC2, scalar2=-1, op0=OP.subtract,
                        op1=OP.mult)
                    for _ in range(1):
                        nc.vector.tensor_mul(t[:], y[:], y[:])
                        nc.vector.tensor_mul(t[:], t[:], hneg[:])
                        nc.vector.scalar_tensor_tensor(
                            out=y[:], in0=t[:], scalar=1.5, in1=y[:],
                            op0=OP.add, op1=OP.mult)
                    nc.vector.tensor_copy(out=rsmr[:, 0:ngr], in_=y[:])
                    # mr = -(sum_x/H)*rstd
                    nc.vector.scalar_tensor_tensor(
                        out=rsmr[:, ngr:2 * ngr], in0=psT[:, 0:ngr],
                        scalar=-1.0 / H, in1=y[:], op0=OP.mult, op1=OP.mult)
                    # transpose each column separately so every row lands
                    # on partition 0 (partition_broadcast requirement)
                    rows = statp.tile([1, 2 * NG * P], F16, tag="rows",
                                      name="rows")
                    for q in range(2 * ngr):
                        ptrq = pst.tile([1, P], F16, tag="pstt", name="ptrq")
                        nc.tensor.transpose(out=ptrq[:], in_=rsmr[:, q:q + 1],
                                            identity=ident16[:])
                        if q % 2 == 0:
                            nc.vector.tensor_copy(
                                out=rows[0:1, q * P:(q + 1) * P], in_=ptrq[:])
                        else:
                            nc.scalar.copy(
                                out=rows[0:1, q * P:(q + 1) * P], in_=ptrq[:])
                    return rows

                def pe_filler(n, rhs_tile):
                    """Keep the PE p-state ramp warm with dead matmuls into
                    a rotating PSUM tile nobody reads."""
                    for _ in range(n):
                        pg = psg.tile([P, 512], F32, tag="pg", name="fill")
                        nc.tensor.matmul(out=pg[:], lhsT=ident16[:],
                                         rhs=rhs_tile[:, 0:512],
                                         start=True, stop=True)

                # ---- phase B: L recurrent layers ----
                for l in range(L):
                    wst = wstp.tile([P, KT * 3 * H], F16, tag="wst")
                    for kk in range(KT):
                        nc.sync.dma_start(
                            out=wst[:, kk * 3 * H:(kk + 1) * 3 * H],
                            in_=wsT_t[l, kk])

                    rows = ln_stats(x, 0, NG, "b")
                    if fa:
                        pe_filler(fa, x[0])

                    # broadcast rstd across partitions (Pool engine); the
                    # -m*rstd mean correction is folded into the gate GEMM as
                    # a rank-1 update (wl x mr) instead of a broadcast.
                    rb = bcp.tile([P, W], F16, tag="rb")
                    for g in range(NG):
                        nc.gpsimd.partition_broadcast(
                            rb[:, g * P:(g + 1) * P],
                            rows[0:1, g * P:(g + 1) * P])

                    # xn = x * rstd
                    xn = [xnp.tile([P, W], F16, tag=f"xn{k}", name=f"xn{k}") for k in range(KT)]
                    for k in range(KT):
                        nc.vector.tensor_mul(xn[k][:], x[k][:], rb[:])
                    if fb:
                        pe_filler(fb, x[0])

                    # --- gates GEMM + nonlinearities + scan ---
                    fp = [scanp.tile([P, W], F16, tag=f"fp{k}", name=f"fp{k}") for k in range(KT)]
                    vv = [scanp.tile([P, W], F16, tag=f"vv{k}", name=f"vv{k}") for k in range(KT)]
                    hh = [scanp.tile([P, W], F16, tag=f"h{k}", name=f"h{k}") for k in range(KT)]
                    x2 = [xpool.tile([P, W], F16, tag=f"x{k}", name=f"xt{k}") for k in range(KT)]
                    for k in range(KT):
                        sf = work.tile([P, W], F16, tag="sf")
                        si = work.tile([P, W], F16, tag="si")
                        sg = work.tile([P, W], F16, tag="sg")
                        lin = work.tile([P, W], F16, tag="lin")
                        for (o, n) in NCH:
                            def gate_mm(gate):
                                pg = psg.tile([P, 512], F32, tag="pg")
                                for kk in range(KT):
                                    c0 = kk * 3 * H + gate * H + k * P
                                    nc.tensor.matmul(
                                        out=pg[:, :n],
                                        lhsT=(wst[:, c0:c0 + P]),
                                        rhs=(xn[kk][:, o:o + n]),
                                        start=(kk == 0), stop=False)
                                w0 = (l * 12 + gate * 4 + k) * P
                                nc.tensor.matmul(
                                    out=pg[:, :n],
                                    lhsT=wlr[0:1, w0:w0 + P],
                                    rhs=rows[0:1, NG * P + o:NG * P + o + n],
                                    start=False, stop=True)
                                return pg

                            pg_f = gate_mm(0)
                            nc.scalar.activation(
                                out=sf[:, o:o + n], in_=pg_f[:, :n],
                                func=AF.Sigmoid,
                                bias=bsg[:, l * 12 + k:l * 12 + k + 1])
                            pg_i = gate_mm(1)
                            nc.scalar.activation(
                                out=si[:, o:o + n], in_=pg_i[:, :n],
                                func=AF.Sigmoid,
                                bias=bsg[:, l * 12 + 4 + k:l * 12 + 4 + k + 1])
                            pg_t = gate_mm(2)
                            nc.scalar.activation(
                                out=sg[:, o:o + n], in_=pg_t[:, :n],
                                func=AF.Sigmoid,
                                bias=bsg[:, l * 12 + 8 + k:l * 12 + 8 + k + 1])
                            nc.scalar.activation(
                                out=lin[:, o:o + n], in_=pg_t[:, :n],
                                func=AF.Identity,
                                bias=bshalf[:, l * 4 + k:l * 4 + k + 1])
                        # full-window gate math (one pass per k)
                        nc.vector._custom_dve(
                            FRAC_OP, out=fp[k][:], in0=sf[:], in1=si[:],
                            s0=FRAC_C0, s1=FRAC_C1)
                        g16 = work.tile([P, W], F16, tag="g16")
                        nc.vector.tensor_max(g16[:], lin[:], sg[:])
                        ip16 = work.tile([P, W], F16, tag="ip16")
                        nc.vector.tensor_scalar(
                            out=ip16[:], in0=fp[k][:],
                            scalar1=-1.0, scalar2=1.0,
                            op0=OP.mult, op1=OP.add)
                        nc.vector.tensor_mul(vv[k][:], ip16[:], g16[:])
                        # boundary reset at own-region start (exact for j==0)
                        t1 = work.tile([P, 1], F32, tag="t1")
                        nc.vector.tensor_mul(
                            t1[:], fp[k][:, HALO:HALO + 1], rst[:, 1:2])
                        nc.vector.tensor_add(
                            vv[k][:, HALO:HALO + 1], t1[:],
                            vv[k][:, HALO:HALO + 1])
                        nc.vector.tensor_mul(
                            fp[k][:, HALO:HALO + 1],
                            fp[k][:, HALO:HALO + 1], rst[:, 0:1])
                        nc.vector.tensor_tensor_scan(
                            out=hh[k][:], data0=fp[k][:], data1=vv[k][:],
                            initial=0.5, op0=OP.mult, op1=OP.add)
                        nc.vector.tensor_add(x2[k][:], hh[k][:], x[k][:])
                    x = x2

                # ---- final LayerNorm (own tokens = groups 1..4) ----
                rows2 = ln_stats(x, 1, NG - 1, "f")
                if fc:
                    pe_filler(fc, x[0])
                rb2 = bcp.tile([P, CHUNK], F16, tag="rb2")
                mb2 = bcp.tile([P, CHUNK], F16, tag="mb2")
                for g in range(NG - 1):
                    nc.gpsimd.partition_broadcast(
                        rb2[:, g * P:(g + 1) * P],
                        rows2[0:1, g * P:(g + 1) * P])
                    nc.gpsimd.partition_broadcast(
                        mb2[:, g * P:(g + 1) * P],
                        rows2[0:1, (NG - 1 + g) * P:(NG + g) * P])
                for k in range(KT):
                    nc.vector.tensor_mul(xf_bf[k][:], x[k][:, HALO:], rb2[:])
                    nc.vector.tensor_add(xf_bf[k][:], xf_bf[k][:], mb2[:])

            # ---- phase C: logits GEMM (own 512 tokens x full vocab) ----
            # fp8e4m3 DoubleRow, 3 residual-corrected passes:
            #   po = W1@X1 + W1@X2 + W3@X1  with W1 = q8(64*w),
            #   W3 = q8(64*w - W1), X1 = q8(xf), X2 = q8(xf - X1);
            #   logits = po/64 + fc_b   (error ~1.2e-3, see prep)
            x1p = [persist.tile([P, 2, CHUNK], F8, tag=f"x1p{i}", name=f"x1p{i}")
                   for i in range(2)]
            x2p = [persist.tile([P, 2, CHUNK], F8, tag=f"x2p{i}", name=f"x2p{i}")
                   for i in range(2)]
            for k in range(KT):
                i, j = divmod(k, 2)
                nc.vector.tensor_copy(out=x1p[i][:, j, :], in_=xf_bf[k][:])
                nc.vector.tensor_sub(x2p[i][:, j, :], xf_bf[k][:],
                                     x1p[i][:, j, :])
            VG = 10   # vocab tiles per fcw load (25 groups of 10)
            DR = mybir.MatmulPerfMode.DoubleRow
            with tc.tile_pool(name="osb", bufs=8) as osbp, \
                 tc.tile_pool(name="pso", bufs=8, space="PSUM") as pso:
                for vg in range(25):
                    fcw = fcwp.tile([P, VG, 2, 2, 2, P], F8, tag="fcw")
                    nc.gpsimd.dma_start(out=fcw[:], in_=fcwt_t[vg])
                    for j in range(VG):
                        vt = vg * VG + j
                        po = pso.tile([P, CHUNK], F32, tag="po")
                        passes = [(0, x1p), (0, x2p), (1, x1p)]
                        nmm = 0
                        for (t, xs) in passes:
                            for i in range(2):
                                nc.tensor.matmul(
                                    out=po[:], lhsT=fcw[:, j, t, i, :, :],
                                    rhs=xs[i][:],
                                    start=(nmm == 0), stop=(nmm == 5),
                                    perf_mode=DR)
                                nmm += 1
                        osb = osbp.tile([P, CHUNK], F16, tag="osb")
                        if vt % 2 == 0:
                            nc.scalar.activation(out=osb[:], in_=po[:],
                                                 func=AF.Identity,
                                                 scale=1.0 / 64,
                                                 bias=fcb2[:, vt:vt + 1])
                        else:
                            nc.vector.tensor_scalar(
                                out=osb[:], in0=po[:],
                                scalar1=1.0 / 64,
                                scalar2=fcb2[:, vt:vt + 1],
                                op0=OP.mult, op1=OP.add)
                        (nc.sync if vt % 2 == 0 else nc.scalar).dma_start(
                            out=out_t[vt * P:(vt + 1) * P, :], in_=osb[:])

    nc.compile()
    return nc


_CACHED = None


def _get_program():
    global _CACHED
    if _CACHED is None:
        _CACHED = build_program()
    return _CACHED


def prep_inputs(ids, emb, Ws, bs, ln_w, ln_b, fln_w, fc_w, fc_b):
    """Host-side layout prep -> per-core input maps."""
    ids = np.asarray(ids)
    emb = np.asarray(emb, dtype=np.float32)
    Ws = np.asarray(Ws, dtype=np.float32)
    bs = np.asarray(bs, dtype=np.float32)
    ln_w = np.asarray(ln_w, dtype=np.float32)
    ln_b = np.asarray(ln_b, dtype=np.float32)
    fln_w = np.asarray(fln_w, dtype=np.float32)
    fc_w = np.asarray(fc_w, dtype=np.float32)
    fc_b = np.asarray(fc_b, dtype=np.float32)

    emb16 = np.ascontiguousarray(emb).astype(np.float16)

    # fold ln_w into the gate weights, ln_b into the gate biases
    # Ws'[l] = Ws[l] * ln_w[l][None,:]; bias'[l] = bs[l] + Ws[l] @ ln_b[l]
    wsT = np.ascontiguousarray(
        np.stack([(Ws[l] * ln_w[l][None, :]).T.reshape(KT, P, 3 * H)
                  for l in range(L)])).astype(np.float16)
    bias = np.stack([bs[l] + Ws[l] @ ln_b[l] for l in range(L)])  # [L, 3H]

    # gate-weight column sums (for the rank-1 mean correction), as a
    # partition-0 row grouped [l][gate][k][128]
    wl = np.stack([(Ws[l] * ln_w[l][None, :]).sum(axis=1) for l in range(L)])
    wlr = np.empty((1, L * 12 * P), np.float32)
    for l in range(L):
        for gate in range(3):
            for k in range(KT):
                c = (l * 12 + gate * 4 + k) * P
                wlr[0, c:c + P] = wl[l, gate * H + k * P:gate * H + (k + 1) * P]

    # per-partition gate biases, grouped [l][gate][k]
    bsg = np.empty((P, L * 12), np.float32)
    bshalf = np.empty((P, L * 4), np.float32)
    for l in range(L):
        for gate in range(3):
            for k in range(KT):
                bsg[:, l * 12 + gate * 4 + k] = \
                    bias[l, gate * H + k * P:gate * H + (k + 1) * P]
        for k in range(KT):
            bshalf[:, l * 4 + k] = bias[l, 2 * H + k * P:2 * H + (k + 1) * P] + 0.5

    # fold fln_w into fc_w; quantize to fp8 e4m3 hi+residual at scale 64,
    # tiled [25, P, 10, 2(hi/res), 2(i), 2(j), P]
    import ml_dtypes
    E4 = ml_dtypes.float8_e4m3
    fcw = fc_w * fln_w[None, :]
    w1 = (64.0 * fcw).astype(E4)
    w3 = (64.0 * fcw - w1.astype(np.float32)).astype(E4)

    def _tile8(w8):
        # [H, V] -> [i 2, j 2, c P, vg 25, vt 10, m P] -> [vg, c, vt, i, j, m]
        return w8.T.reshape(2, 2, P, 25, 10, P).transpose(3, 2, 4, 0, 1, 5)

    fcwt = np.ascontiguousarray(
        np.stack([_tile8(w1), _tile8(w3)], axis=3))
    fcb2 = np.ascontiguousarray(fc_b.reshape(V // P, P).T)

    shared = {"emb": emb16, "wsT": wsT, "bsg": bsg, "bshalf": bshalf,
              "wlr": wlr.astype(np.float16), "fcwt": fcwt, "fcb": fcb2}

    in_maps = []
    for c in range(N_CORES):
        b, j = divmod(c, 4)
        own0 = j * CHUNK
        win = np.zeros(W, np.int32)
        if j == 0:
            win[HALO:] = ids[b, :CHUNK]
        else:
            win[:] = ids[b, own0 - HALO:own0 + CHUNK]
        idxt = np.ascontiguousarray(win.reshape(NG, P).T)
        rstc = np.empty((P, 2), np.float32)
        rstc[:, 0] = 0.0 if j == 0 else 1.0   # multiplies f at window pos HALO
        rstc[:, 1] = 0.5 if j == 0 else 0.0   # adds f*this to v at pos HALO
        in_maps.append({**shared, "idx": idxt, "rst": rstc})
    return in_maps


def kernel(ids, emb, Ws, bs, ln_w, ln_b, fln_w, fc_w, fc_b):
    nc = _get_program()
    in_maps = prep_inputs(ids, emb, Ws, bs, ln_w, ln_b, fln_w, fc_w, fc_b)
    res = run_bass_kernel_spmd(nc, in_maps, list(range(N_CORES)))
    out = np.empty((B, S, V), np.float32)
    for c in range(N_CORES):
        b, j = divmod(c, 4)
        out[b, j * CHUNK:(j + 1) * CHUNK, :] = \
            res.results[c]["out"].T.astype(np.float32)
    return out


# revision 30
# speedup vs baseline: 1.1397x; 1.0721x over previous
"""Trainium2 Bass kernel for a 3-layer minLSTM-style NLP model.

Model (per reference):
  x = emb[ids]                                   (B,S,E) = (2,2048,512)
  3 x { xn = LN(x); gates = xn @ Ws.T + bs;
        f' = sig(f)/(sig(f)+sig(i)); i' = 1-f';
        v = i' * g(tilde), g(x) = max(x+0.5, sigmoid(x));
        h_t = f'_t h_{t-1} + v_t  (h_0 = 0.5);  x = h + x }
  xf = LN(x) * fln_w;  logits = xf @ fc_w.T + fc_b    (B,S,32000)

Sharding (8 cores, zero collectives):
  core c -> (batch b=c//4, seq chunk j=c%4 of 512 tokens). Each core runs a
  640-token window (128-token halo before its own 512) through the recurrent
  stack; the forget-product decays the unknown initial state to ~0 over the
  halo, and a per-core reset constant makes j==0 exact at the batch start.
  Each core computes logits for its own 512 tokens against the full vocab.

Key layout tricks:
  - LN affine (ln_w/ln_b) folded into the gate weights/biases on host, and
    fln_w folded into fc_w, so the device only applies (x-m)*rstd.
  - Per-token LN stats computed TRANSPOSED (tokens on partitions) via 1-row
    matmuls; rsqrt via magic-constant Newton on tiny [128,5] tiles; result
    transposed back and broadcast across partitions by the Pool engine.
  - Activations f16 everywhere; output logits written f16 and upcast on host.
"""

import sys

if "/opt/trn_rl_repo" not in sys.path:
    sys.path.insert(0, "/opt/trn_rl_repo")

import numpy as np

import concourse.bass as bass
import concourse.bacc as bacc
import concourse.tile as tile
from concourse import mybir
from concourse.bass import IndirectOffsetOnAxis
from concourse.bass_utils import run_bass_kernel_spmd
from concourse.masks import make_identity
from concourse import dve_ops as _dve_ops
from concourse.dve_spec import AluOp as _DAlu, Bin as _DBin, Spec as _DSpec, \
    Src0 as _DS0, Src1 as _DS1, C0 as _DC0, C1 as _DC1, lower as _dlower, \
    _has_src1 as _dhas_src1
from concourse.dve_uop import DveOpSpec as _DveOpSpec


def _make_frac_op():
    """Custom DVE op: out = in0 / (in0 + in1), one bit-trick seed + one
    Newton pass (~1.7e-3 rel err). Fuses the minLSTM gate normalization
    f' = sig(f)/(sig(f)+sig(i)) into a single DVE instruction."""
    name = "FRAC_SIG_FAST_ANT"
    for op in _dve_ops.OPS:
        if op.name == name:
            return op
    _z = _DS0 + _DS1
    _nz = _DBin(_DAlu.BITWISE_NOT, _z, _z)
    _y0 = _nz * _DC0
    _y1 = _y0 * (_DC1 - _z * _y0)

    def _ref(in0, in1, c0, c1, c2):
        z = in0.astype(np.float32) + in1.astype(np.float32)
        nz = (~z.view(np.int32)).view(np.float32)
        y0 = nz * np.float32(c0)
        y1 = (y0 * (np.float32(c1) - z * y0)).astype(np.float32)
        return in0.astype(np.float32) * y1

    spec = _DSpec(body=_DS0 * _y1, reference=_ref)
    row = max(_dve_ops._SUB_OPCODE_FOR_NAME.values()) + 1
    _dve_ops._SUB_OPCODE_FOR_NAME[name] = row
    shas = {}
    for ver in ("v3", "v4"):
        u = _dlower(spec, ver=ver)
        shas[ver] = _DveOpSpec(name=name, opcode=row, uops=u,
                               rd1_en=_dhas_src1(spec)).sha(ver)
    op = _dve_ops.DveOp(name, spec, subdim=False, uops_sha=shas,
                        perf_en={"v3": True, "v4": True})
    _dve_ops.OPS.append(op)
    _dve_ops.CUSTOM_DVE_SPECS[name] = spec
    return op


FRAC_OP = _make_frac_op()
FRAC_C0 = -0.23549792
FRAC_C1 = 2.0017324

F32 = mybir.dt.float32
F16 = mybir.dt.float16
F8 = mybir.dt.float8e4
I32 = mybir.dt.int32
AF = mybir.ActivationFunctionType
OP = mybir.AluOpType

# problem constants
B, S, V, H, L = 2, 2048, 32000, 512, 3
P = 128
KT = H // P            # 4 k-tiles over the H contraction dim
CHUNK = 512            # own tokens per core
HALO = 128             # speculative scan warmup tokens
W = HALO + CHUNK       # 640 window tokens per core
NG = W // P            # 5 embedding gather groups
NCH = [(0, 512), (512, 128)]   # window free-dim chunks (PSUM-bounded)
VC = 500               # vocab chunk for logits
N_CORES = 8
EPS = 1e-5
MAGIC2 = 0x1EF759DF    # rsqrt seed magic, pre-adjusted for hneg=-(var+eps)/2


def build_program(fcwb=10, psgb=4, psgrb=1, pstb=2, workb=2, wstb=2, fa=12, fb=8, fc=14):
    nc = bacc.Bacc("TRN2", target_bir_lowering=False, debug=False,
                   enable_asserts=True, num_devices=N_CORES)

    idx_t = nc.dram_tensor("idx", [P, NG], I32, kind="ExternalInput").ap()
    emb_t = nc.dram_tensor("emb", [V, H], F16, kind="ExternalInput").ap()
    wsT_t = nc.dram_tensor("wsT", [L, KT, P, 3 * H], F16, kind="ExternalInput").ap()
    bsg_t = nc.dram_tensor("bsg", [P, L * 12], F32, kind="ExternalInput").ap()
    bshalf_t = nc.dram_tensor("bshalf", [P, L * 4], F32, kind="ExternalInput").ap()
    fcwt_t = nc.dram_tensor("fcwt", [25, P, 10, 2, 2, 2, P], F8, kind="ExternalInput").ap()
    fcb_t = nc.dram_tensor("fcb", [P, V // P], F32, kind="ExternalInput").ap()
    wlr_t = nc.dram_tensor("wlr", [1, L * 12 * P], F16, kind="ExternalInput").ap()
    rst_t = nc.dram_tensor("rst", [P, 2], F32, kind="ExternalInput").ap()
    out_t = nc.dram_tensor("out", [V // (2 * P), P, 2, CHUNK], F16,
                           kind="ExternalOutput").ap()

    with tile.TileContext(nc) as tc:
        with tc.tile_pool(name="singles", bufs=1) as singles, \
             tc.tile_pool(name="persist", bufs=1) as persist, \
             tc.tile_pool(name="fcw", bufs=fcwb) as fcwp:

            # ---- constants / small inputs ----
            idx = singles.tile([P, NG], I32)
            nc.sync.dma_start(out=idx[:], in_=idx_t[:])
            bsg = singles.tile([P, L * 12], F32)
            nc.sync.dma_start(out=bsg[:], in_=bsg_t[:])
            bshalf = singles.tile([P, L * 4], F32)
            nc.sync.dma_start(out=bshalf[:], in_=bshalf_t[:])
            wlr = singles.tile([1, L * 12 * P], F16)
            nc.sync.dma_start(out=wlr[:], in_=wlr_t[:])
            rst = singles.tile([P, 2], F32)
            nc.sync.dma_start(out=rst[:], in_=rst_t[:])
            fcb2 = singles.tile([P, V // P], F32)
            nc.sync.dma_start(out=fcb2[:], in_=fcb_t[:])
            ident16 = singles.tile([P, P], F16)
            make_identity(nc, ident16[:])
            actwarm = singles.tile([1, 1], F32)
            nc.scalar.activation(out=actwarm[:], in_=rst[0:1, 0:1],
                                 func=AF.Sigmoid)
            ones16 = singles.tile([P, 1], F16)   # stats-reduce rhs
            nc.vector.memset(ones16[:], 1.0)

            # final activations (channel-major), consumed by phase C
            xf_bf = [persist.tile([P, CHUNK], F16, tag=f"xfbf{k}", name=f"xfbf{k}")
                     for k in range(KT)]

            with tc.tile_pool(name="xpool", bufs=2) as xpool, \
                 tc.tile_pool(name="wst", bufs=wstb) as wstp, \
                 tc.tile_pool(name="work", bufs=workb) as work, \
                 tc.tile_pool(name="scan", bufs=1) as scanp, \
                 tc.tile_pool(name="xnp", bufs=1) as xnp, \
                 tc.tile_pool(name="gath", bufs=1) as gathp, \
                 tc.tile_pool(name="bc", bufs=1) as bcp, \
                 tc.tile_pool(name="stat", bufs=1) as statp, \
                 tc.tile_pool(name="psg", bufs=psgb, space="PSUM") as psg, \
                 tc.tile_pool(name="pss", bufs=1, space="PSUM") as pss, \
                 tc.tile_pool(name="psgr", bufs=psgrb, space="PSUM") as psgr, \
                 tc.tile_pool(name="pst", bufs=pstb, space="PSUM") as pst:

                # ---- phase A: embedding gather + transpose to channel-major
                x = [xpool.tile([P, W], F16, tag=f"x{k}", name=f"xt{k}") for k in range(KT)]
                xgs = []
                for g in range(NG):
                    xg = gathp.tile([P, H], F16, tag=f"xg{g}", name=f"xg{g}")
                    nc.gpsimd.indirect_dma_start(
                        out=xg[:], out_offset=None, in_=emb_t[:],
                        in_offset=IndirectOffsetOnAxis(ap=idx[:, g:g + 1], axis=0),
                    )
                    xgs.append(xg)
                for g in range(NG):
                    xg = xgs[g]
                    for k in range(KT):
                        ptr = pst.tile([P, P], F16, tag="pstt", name="ptr")
                        nc.tensor.transpose(
                            out=ptr[:], in_=xg[:, k * P:(k + 1) * P],
                            identity=ident16[:])
                        eng = nc.vector if (g * KT + k) % 2 == 0 else nc.scalar
                        if eng is nc.vector:
                            nc.vector.tensor_copy(
                                out=x[k][:, g * P:(g + 1) * P], in_=ptr[:])
                        else:
                            nc.scalar.copy(
                                out=x[k][:, g * P:(g + 1) * P], in_=ptr[:])

                # ---- helper: transposed LN stats + rsqrt newton ----
                def ln_stats(xs, g0, ngr, tag):
                    """Per-token -mean*rstd and rstd for token groups
                    [g0, g0+ngr) of the window, returned as a [1, 2*ngr*P]
                    f16 row on partition 0: cols [0,ngr*P) = rstd,
                    [ngr*P, 2*ngr*P) = mr."""
                    psT = pss.tile([P, 8], F32, tag="psT", name="psT")
                    sums_b = statp.tile([P, ngr], F32, tag="sumb")
                    scr = statp.tile([P, P], F32, tag="ttrscr")
                    for g in range(ngr):
                        sl = slice((g0 + g) * P, (g0 + g + 1) * P)
                        for k in range(KT):
                            nc.tensor.matmul(
                                out=psT[:, g:g + 1], lhsT=xs[k][:, sl],
                                rhs=ones16[:],
                                start=(k == 0), stop=(k == KT - 1))
                        gram = psgr.tile([P, P], F32, tag="gram", name="gram")
                        for k in range(KT):
                            nc.tensor.matmul(
                                out=gram[:], lhsT=xs[k][:, sl],
                                rhs=xs[k][:, sl],
                                start=(k == 0), stop=(k == KT - 1))
                        # sum_x2 = diag(gram) via (gram * I) row-reduce
                        nc.vector.tensor_tensor(
                            out=scr[:], in0=gram[:], in1=ident16[:],
                            op=OP.mult)
                        nc.vector.tensor_reduce(
                            out=sums_b[:, g:g + 1], in_=scr[:], op=OP.add,
                            axis=mybir.AxisListType.X)
                    m2 = statp.tile([P, ngr], F32, tag="m2")
                    hneg = statp.tile([P, ngr], F32, tag="hneg")
                    y = statp.tile([P, ngr], F32, tag="y")
                    t = statp.tile([P, ngr], F32, tag="t")
                    rsmr = statp.tile([P, 2 * ngr], F16, tag="rsmr")
                    # m2 = (sum_x/(H*sqrt2))^2 = m^2/2
                    nc.scalar.activation(out=m2[:], in_=psT[:, 0:ngr],
                                         func=AF.Square,
                                         scale=1.0 / (H * np.sqrt(2.0)))
                    # hneg = m^2/2 - (sum_x2/(2H) + eps/2) = -(var+eps)/2
                    nc.vector.tensor_scalar(
                        out=hneg[:], in0=sums_b[:],
                        scalar1=0.5 / H, scalar2=EPS / 2,
                        op0=OP.mult, op1=OP.add)
                    nc.vector.tensor_sub(hneg[:], m2[:], hneg[:])
                    # rsqrt seed: y = -( (bits(hneg)>>1) - MAGIC2 )
                    nc.vector.tensor_scalar(
                        out=y[:].bitcast(I32), in0=hneg[:].bitcast(I32),
                        scalar1=1, scalar2=None,
                        op0=OP.arith_shift_right)
                    nc.vector.tensor_scalar(
                        out=y[:].bitcast(I32), in0=y[:].bitcast(I32),
                        scalar1=MAGIC2, scalar2=-1, op0=OP.subtract,
                        op1=OP.mult)
                    for _ in range(1):
                        nc.vector.tensor_mul(t[:], y[:], y[:])
                        nc.vector.tensor_mul(t[:], t[:], hneg[:])
                        nc.vector.scalar_tensor_tensor(
                            out=y[:], in0=t[:], scalar=1.5, in1=y[:],
                            op0=OP.add, op1=OP.mult)
                    nc.vector.tensor_copy(out=rsmr[:, 0:ngr], in_=y[:])
                    # mr = -(sum_x/H)*rstd
                    nc.vector.scalar_tensor_tensor(
                        out=rsmr[:, ngr:2 * ngr], in0=psT[:, 0:ngr],
                        scalar=-1.0 / H, in1=y[:], op0=OP.mult, op1=OP.mult)
                    # transpose each column separately so every row lands
                    # on partition 0 (partition_broadcast requirement)
                    rows = statp.tile([1, 2 * NG * P], F16, tag="rows",
                                      name="rows")
                    for q in range(2 * ngr):
                        ptrq = pst.tile([1, P], F16, tag="pstt", name="ptrq")
                        nc.tensor.transpose(out=ptrq[:], in_=rsmr[:, q:q + 1],
                                            identity=ident16[:])
                        if q % 2 == 0:
                            nc.vector.tensor_copy(
                                out=rows[0:1, q * P:(q + 1) * P], in_=ptrq[:])
                        else:
                            nc.scalar.copy(
                                out=rows[0:1, q * P:(q + 1) * P], in_=ptrq[:])
                    return rows

                def pe_filler(n, rhs_tile):
                    """Keep the PE p-state ramp warm with dead matmuls into
                    a rotating PSUM tile nobody reads."""
                    for _ in range(n):
                        pg = psg.tile([P, 512], F32, tag="pg", name="fill")
                        nc.tensor.matmul(out=pg[:], lhsT=ident16[:],
                                         rhs=rhs_tile[:, 0:512],
                                         start=True, stop=True)

                # ---- phase B: L recurrent layers ----
                for l in range(L):
                    wst = wstp.tile([P, KT * 3 * H], F16, tag="wst")
                    for kk in range(KT):
                        nc.sync.dma_start(
                            out=wst[:, kk * 3 * H:(kk + 1) * 3 * H],
                            in_=wsT_t[l, kk])

                    rows = ln_stats(x, 0, NG, "b")
                    if fa:
                        pe_filler(fa, x[0])

                    # broadcast rstd across partitions (Pool engine); the
                    # -m*rstd mean correction is folded into the gate GEMM as
                    # a rank-1 update (wl x mr) instead of a broadcast.
                    rb = bcp.tile([P, W], F16, tag="rb")
                    for g in range(NG):
                        nc.gpsimd.partition_broadcast(
                            rb[:, g * P:(g + 1) * P],
                            rows[0:1, g * P:(g + 1) * P])

                    # xn = x * rstd
                    xn = [xnp.tile([P, W], F16, tag=f"xn{k}", name=f"xn{k}") for k in range(KT)]
                    for k in range(KT):
                        nc.vector.tensor_mul(xn[k][:], x[k][:], rb[:])
                    if fb:
                        pe_filler(fb, x[0])

                    # --- gates GEMM + nonlinearities + scan ---
                    fp = [scanp.tile([P, W], F16, tag=f"fp{k}", name=f"fp{k}") for k in range(KT)]
                    vv = [scanp.tile([P, W], F16, tag=f"vv{k}", name=f"vv{k}") for k in range(KT)]
                    hh = [scanp.tile([P, W], F16, tag=f"h{k}", name=f"h{k}") for k in range(KT)]
                    x2 = [xpool.tile([P, W], F16, tag=f"x{k}", name=f"xt{k}") for k in range(KT)]
                    for k in range(KT):
                        sf = work.tile([P, W], F16, tag="sf")
                        si = work.tile([P, W], F16, tag="si")
                        sg = work.tile([P, W], F16, tag="sg")
                        lin = work.tile([P, W], F16, tag="lin")
                        for (o, n) in NCH:
                            def gate_mm(gate):
                                pg = psg.tile([P, 512], F32, tag="pg")
                                for kk in range(KT):
                                    c0 = kk * 3 * H + gate * H + k * P
                                    nc.tensor.matmul(
                                        out=pg[:, :n],
                                        lhsT=(wst[:, c0:c0 + P]),
                                        rhs=(xn[kk][:, o:o + n]),
                                        start=(kk == 0), stop=False)
                                w0 = (l * 12 + gate * 4 + k) * P
                                nc.tensor.matmul(
                                    out=pg[:, :n],
                                    lhsT=wlr[0:1, w0:w0 + P],
                                    rhs=rows[0:1, NG * P + o:NG * P + o + n],
                                    start=False, stop=True)
                                return pg

                            pg_f = gate_mm(0)
                            nc.scalar.activation(
                                out=sf[:, o:o + n], in_=pg_f[:, :n],
                                func=AF.Sigmoid,
                                bias=bsg[:, l * 12 + k:l * 12 + k + 1])
                            pg_i = gate_mm(1)
                            nc.scalar.activation(
                                out=si[:, o:o + n], in_=pg_i[:, :n],
                                func=AF.Sigmoid,
                                bias=bsg[:, l * 12 + 4 + k:l * 12 + 4 + k + 1])
                            pg_t = gate_mm(2)
                            nc.scalar.activation(
                                out=sg[:, o:o + n], in_=pg_t[:, :n],
                                func=AF.Sigmoid,
                                bias=bsg[:, l * 12 + 8 + k:l * 12 + 8 + k + 1])
                            nc.scalar.activation(
                                out=lin[:, o:o + n], in_=pg_t[:, :n],
                                func=AF.Identity,
                                bias=bshalf[:, l * 4 + k:l * 4 + k + 1])
                        # full-window gate math (one pass per k)
                        nc.vector._custom_dve(
                            FRAC_OP, out=fp[k][:], in0=sf[:], in1=si[:],
                            s0=FRAC_C0, s1=FRAC_C1)
                        g16 = work.tile([P, W], F16, tag="g16")
                        nc.vector.tensor_max(g16[:], lin[:], sg[:])
                        ip16 = work.tile([P, W], F16, tag="ip16")
                        nc.vector.tensor_scalar(
                            out=ip16[:], in0=fp[k][:],
                            scalar1=-1.0, scalar2=1.0,
                            op0=OP.mult, op1=OP.add)
                        nc.vector.tensor_mul(vv[k][:], ip16[:], g16[:])
                        # boundary reset at own-region start (exact for j==0)
                        t1 = work.tile([P, 1], F32, tag="t1")
                        nc.vector.tensor_mul(
                            t1[:], fp[k][:, HALO:HALO + 1], rst[:, 1:2])
                        nc.vector.tensor_add(
                            vv[k][:, HALO:HALO + 1], t1[:],
                            vv[k][:, HALO:HALO + 1])
                        nc.vector.tensor_mul(
                            fp[k][:, HALO:HALO + 1],
                            fp[k][:, HALO:HALO + 1], rst[:, 0:1])
                        nc.vector.tensor_tensor_scan(
                            out=hh[k][:], data0=fp[k][:], data1=vv[k][:],
                            initial=0.5, op0=OP.mult, op1=OP.add)
                        nc.vector.tensor_add(x2[k][:], hh[k][:], x[k][:])
                    x = x2

                # ---- final LayerNorm (own tokens = groups 1..4) ----
                rows2 = ln_stats(x, 1, NG - 1, "f")
                if fc:
                    pe_filler(fc, x[0])
                rb2 = bcp.tile([P, CHUNK], F16, tag="rb2")
                mb2 = bcp.tile([P, CHUNK], F16, tag="mb2")
                for g in range(NG - 1):
                    nc.gpsimd.partition_broadcast(
                        rb2[:, g * P:(g + 1) * P],
                        rows2[0:1, g * P:(g + 1) * P])
                    nc.gpsimd.partition_broadcast(
                        mb2[:, g * P:(g + 1) * P],
                        rows2[0:1, (NG - 1 + g) * P:(NG + g) * P])
                for k in range(KT):
                    nc.vector.tensor_mul(xf_bf[k][:], x[k][:, HALO:], rb2[:])
                    nc.vector.tensor_add(xf_bf[k][:], xf_bf[k][:], mb2[:])

            # ---- phase C: logits GEMM (own 512 tokens x full vocab) ----
            # fp8e4m3 DoubleRow, 3 residual-corrected passes:
            #   po = W1@X1 + W1@X2 + W3@X1  with W1 = q8(64*w),
            #   W3 = q8(64*w - W1), X1 = q8(xf), X2 = q8(xf - X1);
            #   logits = po/64 + fc_b   (error ~1.2e-3, see prep)
            x1p = [persist.tile([P, 2, CHUNK], F8, tag=f"x1p{i}", name=f"x1p{i}")
                   for i in range(2)]
            x2p = [persist.tile([P, 2, CHUNK], F8, tag=f"x2p{i}", name=f"x2p{i}")
                   for i in range(2)]
            for k in range(KT):
                i, j = divmod(k, 2)
                nc.vector.tensor_copy(out=x1p[i][:, j, :], in_=xf_bf[k][:])
                nc.vector.tensor_sub(x2p[i][:, j, :], xf_bf[k][:],
                                     x1p[i][:, j, :])
            VG = 10   # vocab tiles per fcw load (25 groups of 10)
            DR = mybir.MatmulPerfMode.DoubleRow
            with tc.tile_pool(name="osb", bufs=8) as osbp, \
                 tc.tile_pool(name="pso", bufs=8, space="PSUM") as pso:
                for vg in range(25):
                    fcw = fcwp.tile([P, VG, 2, 2, 2, P], F8, tag="fcw")
                    nc.gpsimd.dma_start(out=fcw[:], in_=fcwt_t[vg])
                    for j in range(VG):
                        vt = vg * VG + j
                        po = pso.tile([P, CHUNK], F32, tag="po")
                        passes = [(0, x1p), (0, x2p), (1, x1p)]
                        nmm = 0
                        for (t, xs) in passes:
                            for i in range(2):
                                nc.tensor.matmul(
                                    out=po[:], lhsT=fcw[:, j, t, i, :, :],
                                    rhs=xs[i][:],
                                    start=(nmm == 0), stop=(nmm == 5),
                                    perf_mode=DR)
                                nmm += 1
                        if vt % 2 == 0:
                            osb = osbp.tile([P, 2, CHUNK], F16, tag="osb")
                            nc.scalar.activation(out=osb[:, 0, :], in_=po[:],
                                                 func=AF.Identity,
                                                 scale=1.0 / 64,
                                                 bias=fcb2[:, vt:vt + 1])
                        else:
                            nc.vector.tensor_scalar(
                                out=osb[:, 1, :], in0=po[:],
                                scalar1=1.0 / 64,
                                scalar2=fcb2[:, vt:vt + 1],
                                op0=OP.mult, op1=OP.add)
                            (nc.sync if (vt // 2) % 2 == 0
                             else nc.scalar).dma_start(
                                out=out_t[vt // 2], in_=osb[:])

    nc.compile()
    return nc


_CACHED = None


def _get_program():
    global _CACHED
    if _CACHED is None:
        _CACHED = build_program()
    return _CACHED


def prep_inputs(ids, emb, Ws, bs, ln_w, ln_b, fln_w, fc_w, fc_b):
    """Host-side layout prep -> per-core input maps."""
    ids = np.asarray(ids)
    emb = np.asarray(emb, dtype=np.float32)
    Ws = np.asarray(Ws, dtype=np.float32)
    bs = np.asarray(bs, dtype=np.float32)
    ln_w = np.asarray(ln_w, dtype=np.float32)
    ln_b = np.asarray(ln_b, dtype=np.float32)
    fln_w = np.asarray(fln_w, dtype=np.float32)
    fc_w = np.asarray(fc_w, dtype=np.float32)
    fc_b = np.asarray(fc_b, dtype=np.float32)

    emb16 = np.ascontiguousarray(emb).astype(np.float16)

    # fold ln_w into the gate weights, ln_b into the gate biases
    # Ws'[l] = Ws[l] * ln_w[l][None,:]; bias'[l] = bs[l] + Ws[l] @ ln_b[l]
    wsT = np.ascontiguousarray(
        np.stack([(Ws[l] * ln_w[l][None, :]).T.reshape(KT, P, 3 * H)
                  for l in range(L)])).astype(np.float16)
    bias = np.stack([bs[l] + Ws[l] @ ln_b[l] for l in range(L)])  # [L, 3H]

    # gate-weight column sums (for the rank-1 mean correction), as a
    # partition-0 row grouped [l][gate][k][128]
    wl = np.stack([(Ws[l] * ln_w[l][None, :]).sum(axis=1) for l in range(L)])
    wlr = np.empty((1, L * 12 * P), np.float32)
    for l in range(L):
        for gate in range(3):
            for k in range(KT):
                c = (l * 12 + gate * 4 + k) * P
                wlr[0, c:c + P] = wl[l, gate * H + k * P:gate * H + (k + 1) * P]

    # per-partition gate biases, grouped [l][gate][k]
    bsg = np.empty((P, L * 12), np.float32)
    bshalf = np.empty((P, L * 4), np.float32)
    for l in range(L):
        for gate in range(3):
            for k in range(KT):
                bsg[:, l * 12 + gate * 4 + k] = \
                    bias[l, gate * H + k * P:gate * H + (k + 1) * P]
        for k in range(KT):
            bshalf[:, l * 4 + k] = bias[l, 2 * H + k * P:2 * H + (k + 1) * P] + 0.5

    # fold fln_w into fc_w; quantize to fp8 e4m3 hi+residual at scale 64,
    # tiled [25, P, 10, 2(hi/res), 2(i), 2(j), P]
    import ml_dtypes
    E4 = ml_dtypes.float8_e4m3
    fcw = fc_w * fln_w[None, :]
    w1 = (64.0 * fcw).astype(E4)
    w3 = (64.0 * fcw - w1.astype(np.float32)).astype(E4)

    def _tile8(w8):
        # [H, V] -> [i 2, j 2, c P, vg 25, vt 10, m P] -> [vg, c, vt, i, j, m]
        return w8.T.reshape(2, 2, P, 25, 10, P).transpose(3, 2, 4, 0, 1, 5)

    fcwt = np.ascontiguousarray(
        np.stack([_tile8(w1), _tile8(w3)], axis=3))
    fcb2 = np.ascontiguousarray(fc_b.reshape(V // P, P).T)

    shared = {"emb": emb16, "wsT": wsT, "bsg": bsg, "bshalf": bshalf,
              "wlr": wlr.astype(np.float16), "fcwt": fcwt, "fcb": fcb2}

    in_maps = []
    for c in range(N_CORES):
        b, j = divmod(c, 4)
        own0 = j * CHUNK
        win = np.zeros(W, np.int32)
        if j == 0:
            win[HALO:] = ids[b, :CHUNK]
        else:
            win[:] = ids[b, own0 - HALO:own0 + CHUNK]
        idxt = np.ascontiguousarray(win.reshape(NG, P).T)
        rstc = np.empty((P, 2), np.float32)
        rstc[:, 0] = 0.0 if j == 0 else 1.0   # multiplies f at window pos HALO
        rstc[:, 1] = 0.5 if j == 0 else 0.0   # adds f*this to v at pos HALO
        in_maps.append({**shared, "idx": idxt, "rst": rstc})
    return in_maps


def kernel(ids, emb, Ws, bs, ln_w, ln_b, fln_w, fc_w, fc_b):
    nc = _get_program()
    in_maps = prep_inputs(ids, emb, Ws, bs, ln_w, ln_b, fln_w, fc_w, fc_b)
    res = run_bass_kernel_spmd(nc, in_maps, list(range(N_CORES)))
    out = np.empty((B, S, V), np.float32)
    for c in range(N_CORES):
        b, j = divmod(c, 4)
        arr = res.results[c]["out"]  # [125, P, 2, CHUNK]
        out[b, j * CHUNK:(j + 1) * CHUNK, :] = \
            arr.transpose(3, 0, 2, 1).reshape(CHUNK, V).astype(np.float32)
    return out
